# revision 4
# baseline (speedup 1.0000x reference)
"""MultiHeadAttention Trainium2 kernel.

Sharding: 8 cores = 4 batches x 2 head-groups (8 heads each).
Each core computes, for its (batch b, head-group g):
  Q^T = Wq_g @ Xq^T, K^T = Wk_g @ Xk^T   (bf16 inputs/weights, f32 PSUM,
  [headdim, S] layout), V = Xv @ Wv_g^T  ([S, 512] layout, +ones col,
  mask-scaled, bf16), scores^T[k,q] per head (K=64 matmuls),
  e = exp(s/8) on ACT (PSUM->SBUF, f32r).
  attnV runs with e as the STATIONARY operand and V as the 65-wide moving
  operand (x~[q, hd] += e_chunk^T-weighted V), so the PE pays 65 cols
  instead of 512 per (head, k-tile, q-chunk): the softmax denominator is
  the ones column and lands per-PARTITION, so normalization is a plain
  per-partition reciprocal + scalar multiply on DVE.  The normalized
  x~[q, hd] staging tile is transposed back to [hd, q] layout with a
  cheap DMA xbar transpose (SP/HWDGE/DMA engines, zero PE cost), then
  out^T_partial = Wo_g^T.T @ x^T (bf16).
Host sums the two head-group partials per batch and transposes back.

Mask handling: V rows and the ones column are multiplied by mask (0/1), which
masks both the attnV numerator and the softmax denominator exactly.

DMA traffic runs in bf16 (inputs, weights, out partials) and is batched into
whole-block transfers (the descriptor engine costs ~625ns per DMA, so many
small DMAs serialize); PSUM accumulation stays f32 and exp output stays f32r.
"""
import contextlib
import os

import numpy as np
import ml_dtypes
import concourse.bass as bass  # noqa: F401
import concourse.tile as tile
from concourse import bacc, mybir
from concourse.bass_utils import run_bass_kernel_spmd

F32 = mybir.dt.float32
F32R = mybir.dt.float32r
BF16 = mybir.dt.bfloat16
EXP = mybir.ActivationFunctionType.Exp

B, S, DM = 4, 2048, 1024
H = 16
DK = 64
HLOC = 8              # heads per core
CW = HLOC * DK        # 512 local head dims per core
NC_CORES = 8
KT = S // 128         # 16 k-tiles
NB = S // 512         # 4 q/s blocks of 512
MT = CW // 128        # 4 m-tiles of local head dims
DT = DM // 128        # 8 contraction tiles over d_model
SCALE = 1.0 / np.sqrt(DK)

_NC = None


def _env(k, d):
    return int(os.environ.get(k, d))


def _build():
    nc = bacc.Bacc()
    xqT = nc.dram_tensor("xqT", [DM, S], BF16, kind="ExternalInput")
    xkT = nc.dram_tensor("xkT", [DM, S], BF16, kind="ExternalInput")
    xvT = nc.dram_tensor("xvT", [DM, S], BF16, kind="ExternalInput")
    wqT = nc.dram_tensor("wqT", [DM, CW], BF16, kind="ExternalInput")
    wkT = nc.dram_tensor("wkT", [DM, CW], BF16, kind="ExternalInput")
    wvT = nc.dram_tensor("wvT", [DM, CW], BF16, kind="ExternalInput")
    woT = nc.dram_tensor("woT", [CW, DM], BF16, kind="ExternalInput")
    maskf = nc.dram_tensor("maskf", [128, KT], F32, kind="ExternalInput")
    outT = nc.dram_tensor("outT", [DM, S], BF16, kind="ExternalOutput")

    # DRAM views with the k-tile dim split out: row (k*128+p) -> [p, k, cols]
    xqv = xqT.rearrange("(k p) s -> p k s", p=128)
    xkv = xkT.rearrange("(k p) s -> p k s", p=128)
    xvv = xvT.rearrange("(k p) s -> p k s", p=128)
    wqv = wqT.rearrange("(k p) c -> p k c", p=128)
    wkv = wkT.rearrange("(k p) c -> p k c", p=128)
    wvv = wvT.rearrange("(k p) c -> p k c", p=128)
    wov = woT.rearrange("(k p) c -> p k c", p=128)
    outv = outT.rearrange("(m p) s -> p m s", p=128)

    with tile.TileContext(nc) as tc, contextlib.ExitStack() as ctx:
        persist = ctx.enter_context(tc.tile_pool(name="persist", bufs=1))

        # --- persistent tiles: mask, wo, Q^T/K^T slices, V ---
        m_sb = persist.tile([128, KT], F32)
        nc.sync.dma_start(m_sb[:], maskf[:])
        ones8 = persist.tile([128, HLOC], F32)
        nc.vector.memset(ones8[:], 1.0)
        warm = persist.tile([1, 1], F32)
        nc.scalar.activation(warm[:], ones8[0:1, 0:1], EXP, scale=1.0)
        q_tiles = {}   # (m, nb) -> [128, 512] bf16  (Q^T slice)
        k_tiles = {}
        for m in range(MT):
            for n in range(NB):
                q_tiles[(m, n)] = persist.tile(
                    [128, 512], BF16, tag=f"q{m}_{n}", name=f"q{m}_{n}")
                k_tiles[(m, n)] = persist.tile(
                    [128, 512], BF16, tag=f"k{m}_{n}", name=f"k{m}_{n}")
        v_sb = persist.tile([128, KT, HLOC, DK + 1], BF16, tag="v")
        wo_t = persist.tile([128, MT, DM], BF16, tag="wo")

        # ---------------- Phase A: projections ----------------
        wq_pool = ctx.enter_context(tc.tile_pool(name="wqp", bufs=1))
        xt = ctx.enter_context(tc.tile_pool(name="xt", bufs=_env("K_XT_BUFS", 6)))
        ctxA = contextlib.ExitStack()
        with ctxA:
            wkv_pool = ctxA.enter_context(tc.tile_pool(name="wkv", bufs=1))
            psA = ctxA.enter_context(tc.tile_pool(name="psA", bufs=8, space="PSUM"))
            wq_sb = wq_pool.tile([128, DT, CW], BF16, tag="wq")
            wk_sb = wkv_pool.tile([128, DT, CW], BF16, tag="wk")
            wv_sb = wq_pool.tile([128, DT, CW], BF16, tag="wv")

            def dma_block(srcv, n, nm, halves=False):
                """One batched DMA (or two halves) for an x block: returns
                [128, DT, 512] bf16 tile."""
                xts = xt.tile([128, DT, 512], BF16, tag="xt", name=f"{nm}{n}")
                cs = slice(n * 512, (n + 1) * 512)
                if halves:
                    h = DT // 2
                    nc.sync.dma_start(xts[:, 0:h, :], srcv[:, 0:h, cs])
                    nc.sync.dma_start(xts[:, h:DT, :], srcv[:, h:DT, cs])
                else:
                    nc.sync.dma_start(xts[:], srcv[:, :, cs])
                return xts

            # k-major projection block: 4 PSUM groups accumulate in lockstep
            # so the first matmul only waits on the first half-DMAs.
            def proj_block_kmajor(dst_tiles, w_sb, xts, n, nm,
                                  split_evac=False, mlist=None):
                mlist = list(range(MT)) if mlist is None else mlist
                ps = {m: psA.tile([128, 512], F32, tag="pa",
                                  name=f"pj{nm}{n}_{m}") for m in mlist}
                for k in range(DT):
                    for m in mlist:
                        nc.tensor.matmul(
                            ps[m][:], w_sb[:, k, m * 128:(m + 1) * 128],
                            xts[:, k, :], start=(k == 0), stop=(k == DT - 1))
                for m in mlist:
                    if split_evac and m % 2:
                        nc.scalar.copy(dst_tiles[(m, n)][:], ps[m][:])
                    else:
                        nc.vector.tensor_copy(dst_tiles[(m, n)][:], ps[m][:])

            # single projection group (phase-B side work; DMAs long done)
            def proj_group(dst_tiles, w_sb, xts, n, m, pool, tag):
                ps = pool.tile([128, 512], F32, tag=tag, name=f"pj{n}_{m}_{tag}")
                for k in range(DT):
                    nc.tensor.matmul(
                        ps[:], w_sb[:, k, m * 128:(m + 1) * 128],
                        xts[:, k, :], start=(k == 0), stop=(k == DT - 1))
                nc.vector.tensor_copy(dst_tiles[(m, n)][:], ps[:])

            def v_evac(n, sm, ps):
                t = n * 4 + sm
                nc.vector.tensor_scalar_mul(
                    v_sb[:, t, :, 0:DK],
                    ps[:].rearrange("p (h d) -> p h d", h=HLOC),
                    m_sb[:, t:t + 1])
                nc.vector.tensor_scalar_mul(
                    v_sb[:, t, :, DK:DK + 1], ones8[:],
                    m_sb[:, t:t + 1])

            def v_block_kmajor(n, xts):
                ps = [psA.tile([128, 512], F32, tag="pa",
                               name=f"vps{n}_{sm}") for sm in range(4)]
                for k in range(DT):
                    for sm in range(4):
                        nc.tensor.matmul(
                            ps[sm][:], xts[:, k, sm * 128:(sm + 1) * 128],
                            wv_sb[:, k, :], start=(k == 0), stop=(k == DT - 1))
                for sm in range(4):
                    v_evac(n, sm, ps[sm])

            def v_group(n, sm, xts, pool, tag):
                ps = pool.tile([128, 512], F32, tag=tag, name=f"vps{n}_{sm}")
                for k in range(DT):
                    nc.tensor.matmul(
                        ps[:], xts[:, k, sm * 128:(sm + 1) * 128],
                        wv_sb[:, k, :], start=(k == 0), stop=(k == DT - 1))
                v_evac(n, sm, ps)

            # Phase-A DMA issue order = consumption order.
            hh = DT // 2
            qq = DT // 4
            nc.sync.dma_start(wk_sb[:, 0:qq, :], wkv[:, 0:qq, :])
            xk0 = xt.tile([128, DT, 512], BF16, tag="xt", name="xk0")
            nc.sync.dma_start(xk0[:, 0:qq, :], xkv[:, 0:qq, 0:512])
            nc.sync.dma_start(wk_sb[:, qq:hh, :], wkv[:, qq:hh, :])
            nc.sync.dma_start(xk0[:, qq:hh, :], xkv[:, qq:hh, 0:512])
            nc.sync.dma_start(wk_sb[:, hh:DT, :], wkv[:, hh:DT, :])
            nc.sync.dma_start(xk0[:, hh:DT, :], xkv[:, hh:DT, 0:512])
            xk_blocks = [xk0] + [dma_block(xkv, n, "xk") for n in range(1, NB)]
            nc.sync.dma_start(wq_sb[:], wqv[:])
            xq0 = dma_block(xqv, 0, "xq")
            nc.sync.dma_start(wv_sb[:], wvv[:])
            xv0 = dma_block(xvv, 0, "xv")
            xv1 = dma_block(xvv, 1, "xv")
            nc.sync.dma_start(wo_t[:], wov[:])

            # PE warmup: dummy matmuls cover initial DMA latency and start
            # the HAM activity window before the first real matmul.
            dum = wq_pool.tile([128, 512], BF16, tag="dum")
            nc.gpsimd.memset(dum[:], 0.0)
            for i in range(_env("K_WARM_MM", 2)):
                pw = psA.tile([128, 512], F32, tag="pa", name=f"warmmm{i}")
                for rep in range(_env("K_WARM_REP", 5)):
                    nc.tensor.matmul(pw[:], dum[:, 0:128], dum[:],
                                     start=(rep == 0), stop=True)
            for n in range(NB):
                proj_block_kmajor(k_tiles, wk_sb, xk_blocks[n], n, "xk")
            v_block_kmajor(0, xv0)
            v_block_kmajor(1, xv1)
            xv2 = dma_block(xvv, 2, "xv")
            xv3 = dma_block(xvv, 3, "xv")
            proj_block_kmajor(q_tiles, wq_sb, xq0, 0, "xq",
                              split_evac=True, mlist=[0, 1])

        # ---------------- Phase B: attention + out-proj ----------------
        # q blocks: three 512-wide (SGW=2), two 256-wide (SGW=4) so the
        # serial final out-projection tail is halved. Narrow blocks keep the
        # exp instruction count low by covering 4 k-tiles per activation.
        QB = [(0, 512, 2), (512, 512, 2), (1024, 512, 2),
              (1536, 256, 4), (1792, 256, 4)]
        NQB = len(QB)
        with tc.tile_pool(name="ev", bufs=_env("K_EV_BUFS", 4)) as ev, \
             tc.tile_pool(name="x", bufs=2) as xpool, \
             tc.tile_pool(name="xn", bufs=2) as xnpool, \
             tc.tile_pool(name="small", bufs=_env("K_SMALL_BUFS", 4)) as small, \
             tc.tile_pool(name="o", bufs=2) as opool, \
             tc.tile_pool(name="psS", bufs=_env("K_PSS_BUFS", 3), space="PSUM") as psS, \
             tc.tile_pool(name="psX", bufs=_env("K_XO_BUFS", 2), space="PSUM") as psX:
            x_tiles = [xpool.tile([128, MT, 512], BF16, tag="xs",
                                  name=f"xs{i}") for i in range(2)]
            xn_stage = [xnpool.tile([128, 4, 512], BF16, tag="xn",
                                    name=f"xn{i}") for i in range(2)]
            o_tiles = [opool.tile([128, DT, 512], BF16, tag="ob",
                                  name=f"ob{i}") for i in range(2)]

            def outproj_group(oqb, m, flush=False):
                col0, W, _ = QB[oqb]
                x_prev = x_tiles[oqb % 2]
                o_sb = o_tiles[oqb % 2]
                po = psS.tile([128, W], F32, tag="s", name=f"po{oqb}_{m}")
                for kk in range(MT):
                    nc.tensor.matmul(
                        po[:], wo_t[:, kk, m * 128:(m + 1) * 128],
                        x_prev[:, kk, 0:W], start=(kk == 0), stop=(kk == MT - 1))
                nc.vector.tensor_copy(o_sb[:, m, 0:W], po[:])
                if flush:
                    # batched output DMA for this q block
                    nc.sync.dma_start(
                        outv[:, :, col0:col0 + W], o_sb[:, :, 0:W])

            # side-work: one psS-slot matmul group (or a DMA batch) per sg
            # step. v-block deadline: attnV eats V tile t at emission slot
            # t//SGW+1. Q_n must be complete before q block n starts.
            xts_store = {("v", 2): xv2, ("v", 3): xv3, ("q", 0): xq0}

            def mk_vg(nn, sm):
                return ("mm", lambda: v_group(nn, sm, xts_store[("v", nn)],
                                              psS, "s"))

            def mk_qdma(nn):
                def f():
                    xts_store[("q", nn)] = dma_block(xqv, nn, "xq")
                return ("dma", f)

            def mk_qg(nn, m):
                return ("mm", lambda: proj_group(q_tiles, wq_sb,
                                                 xts_store[("q", nn)],
                                                 nn, m, psS, "s"))

            def mk_og(oqb, m, flush=False):
                return ("mm", lambda: outproj_group(oqb, m, flush))

            # (qb, p) -> [(min_sg, (kind, fn)), ...]
            side_work = {}
            VOFF = _env("K_VOFF", 1)
            side_work[(0, 0)] = [
                (max(0, VOFF + i), mk_vg(2 + i // 4, i % 4)) for i in range(8)]
            # Per-pair balancing: every pair (not just p0) hosts enough side
            # matmul groups to keep PE ahead of the ACT exp stream. Q_n's
            # m-groups spread across the hosting block's pairs (group m is
            # only needed when block n reaches pair m). og of block i may
            # only run while x_tiles[i%2] is intact: anywhere in block i+1,
            # but only in block i+2's p0 early slots.
            side_work[(0, 1)] = [(0, mk_qdma(1)), (3, mk_qg(0, 2)),
                                 (6, mk_qg(1, 0))]
            side_work[(0, 2)] = [(3, mk_qg(0, 3)), (6, mk_qg(1, 1))]
            side_work[(0, 3)] = [(3, mk_qg(1, 2)), (6, mk_qg(1, 3))]
            SIDE = {
                (1, 0): [(0, 'qdma', 2), (1, 'og', 0, 0), (7, 'og', 0, 1),
                         (4, 'qg', 2, 0)],
                (1, 1): [(1, 'og', 0, 2), (7, 'og', 0, 3), (4, 'qg', 2, 1)],
                (1, 2): [(1, 'og', 0, 4), (7, 'og', 0, 5), (4, 'qg', 2, 2)],
                (1, 3): [(1, 'og', 0, 6), (7, 'og', 0, 7), (4, 'qg', 2, 3)],
                (2, 0): [(0, 'qdma', 3), (1, 'og', 1, 0), (7, 'og', 1, 1),
                         (4, 'qg', 3, 0)],
                (2, 1): [(1, 'og', 1, 2), (7, 'og', 1, 3), (4, 'qg', 3, 1)],
                (2, 2): [(1, 'og', 1, 4), (7, 'og', 1, 5), (4, 'qg', 3, 2)],
                (2, 3): [(1, 'og', 1, 6), (7, 'og', 1, 7), (4, 'qg', 3, 3)],
                (3, 0): [(1, 'og', 2, 0)],
                (3, 1): [(1, 'og', 2, 2), (2, 'og', 2, 3)],
                (3, 2): [(1, 'og', 2, 4), (3, 'og', 2, 1)],
                (3, 3): [(1, 'og', 2, 5)],
                (4, 0): [(0, 'og', 2, 6), (2, 'og', 2, 7)],
                (4, 1): [(0, 'og', 3, 0), (1, 'og', 3, 1), (3, 'og', 3, 2)],
                (4, 2): [(0, 'og', 3, 3), (1, 'og', 3, 4), (3, 'og', 3, 5)],
                (4, 3): [(0, 'og', 3, 6), (1, 'og', 3, 7)],
            }
            for key, items in SIDE.items():
                lst = side_work.setdefault(key, [])
                for it in items:
                    if it[1] == 'qdma':
                        lst.append((it[0], mk_qdma(it[2])))
                    elif it[1] == 'qg':
                        lst.append((it[0], mk_qg(it[2], it[3])))
                    else:
                        lst.append((it[0], mk_og(it[2], it[3],
                                                 flush=(it[3] == DT - 1))))
            for key in side_work:
                side_work[key].sort(key=lambda it: it[0])

            MAXMM = _env("K_MAXMM", 1)

            def side_step(qb, p, sg):
                work = side_work.get((qb, p))
                if not work:
                    return
                did_mm = 0
                while work:
                    min_sg, (kind, fn) = work[0]
                    if min_sg > sg or (kind == "mm" and did_mm >= MAXMM):
                        break
                    work.pop(0)
                    fn()
                    if kind == "mm":
                        did_mm += 1

            def side_flush(qb, p):
                for _, (kind, fn) in side_work.pop((qb, p), []):
                    fn()

            def attn_v(ps_x, h, sg, sgw, e_h, nqc):
                """Flipped attnV for supergroup sg: e chunks stationary,
                V [128, 65] moving, accumulating x~[q, hd|den] per qc."""
                for tt in range(sgw):
                    t = sg * sgw + tt
                    for qc in range(nqc):
                        # start=True zeroes the WHOLE psum bank, so only the
                        # very first matmul into this tile may set it; the
                        # other qc groups accumulate onto the zeroed bank.
                        nc.tensor.matmul(
                            ps_x[h][:, qc, :],
                            e_h[:, tt, qc * 128:(qc + 1) * 128],
                            v_sb[:, t, h, :],
                            start=(t == 0 and qc == 0), stop=(t == KT - 1))

            for qb in range(NQB):
                col0, W, sgw = QB[qb]
                nb = col0 // 512
                q0 = col0 % 512
                nsg = KT // sgw
                nqc = W // 128
                xn_sb = xn_stage[qb % 2]
                for p in range(MT):        # head pairs; pair p = heads 2p,2p+1
                    heads = (2 * p, 2 * p + 1)
                    ps_x = {h: psX.tile([128, nqc, DK + 1], F32, tag="xo",
                                        name=f"psx{qb}_{h}") for h in heads}
                    e_prev = None
                    for sg in range(nsg):
                        # side work: outproj of qb-1, V, or late q projection
                        side_step(qb, p, sg)
                        e_new = {}
                        for h in heads:
                            hp = h % 2
                            ps_h = psS.tile([128, sgw, W], F32, tag="s",
                                            name=f"pss{qb}_{sg}_{h}")
                            for tt in range(sgw):
                                t = sg * sgw + tt
                                nc.tensor.matmul(
                                    ps_h[:, tt, :],
                                    k_tiles[(p, t // 4)][
                                        hp * 64:(hp + 1) * 64,
                                        (t % 4) * 128:(t % 4 + 1) * 128],
                                    q_tiles[(p, nb)][hp * 64:(hp + 1) * 64,
                                                     q0:q0 + W],
                                    start=True, stop=True)
                            # hand this head's exp to ACT immediately, then
                            # run its previous-supergroup attnV (1-sg lag)
                            # while the other head's scores stream
                            e_sb = ev.tile([128, sgw, W], BF16, tag="e",
                                           name=f"e{qb}_{sg}_{h}")
                            nc.scalar.activation(e_sb[:], ps_h[:], EXP,
                                                 scale=float(SCALE))
                            e_new[h] = e_sb
                            if e_prev is not None:
                                attn_v(ps_x, h, sg - 1, sgw, e_prev[h], nqc)
                        e_prev = e_new
                    side_flush(qb, p)
                    # drain last supergroup + normalize into xn staging
                    for h in heads:
                        attn_v(ps_x, h, nsg - 1, sgw, e_prev[h], nqc)
                        hp = h % 2
                        c0 = p * 128 + hp * 64
                        for qc in range(nqc):
                            r = small.tile([128, 1], F32, tag="r",
                                           name=f"r{qb}_{h}_{qc}")
                            nc.vector.reciprocal(r[:], ps_x[h][:, qc, DK:DK + 1])
                            nc.vector.tensor_scalar_mul(
                                xn_sb[:, qc, c0:c0 + 64],
                                ps_x[h][:, qc, 0:DK], r[:])
                # x~ staged as [q, hd]; flip back to [hd(pair), q] via the
                # DMA xbar transpose (SP queue + DMA engines, no PE cost)
                for qc in range(nqc):
                    nc.sync.dma_start_transpose(
                        x_tiles[qb % 2][:, :, qc * 128:(qc + 1) * 128],
                        xn_sb[:, qc, :])
            # final out-projection for the last q block (its og side-work
            # can't ride a following block).
            oqb = NQB - 1
            col0, W, _ = QB[oqb]
            o_sb = o_tiles[oqb % 2]
            x_prev = x_tiles[oqb % 2]
            for m in range(DT):
                ms = slice(m * 128, (m + 1) * 128)
                po = psS.tile([128, W], F32, tag="s", name=f"pof{m}")
                for kk in range(MT):
                    nc.tensor.matmul(
                        po[:], wo_t[:, kk, ms], x_prev[:, kk, 0:W],
                        start=(kk == 0), stop=(kk == MT - 1))
                if m % 2:
                    nc.scalar.copy(o_sb[:, m, 0:W], po[:])
                else:
                    nc.vector.tensor_copy(o_sb[:, m, 0:W], po[:])
                if m == 3:
                    nc.sync.dma_start(
                        outv[:, 0:4, col0:col0 + W], o_sb[:, 0:4, 0:W])
                elif m == 6:
                    nc.sync.dma_start(
                        outv[:, 4:7, col0:col0 + W], o_sb[:, 4:7, 0:W])
            nc.sync.dma_start(
                outv[:, 7:8, col0:col0 + W], o_sb[:, 7:8, 0:W])
    nc.finalize()
    return nc


def kernel(query, key, value, mask, W_q, W_k, W_v, W_o):
    global _NC
    if _NC is None:
        _NC = _build()
    bf = ml_dtypes.bfloat16
    query = np.asarray(query, dtype=np.float32)
    key = np.asarray(key, dtype=np.float32)
    value = np.asarray(value, dtype=np.float32)
    W_q = np.asarray(W_q, dtype=np.float32)
    W_k = np.asarray(W_k, dtype=np.float32)
    W_v = np.asarray(W_v, dtype=np.float32)
    W_o = np.asarray(W_o, dtype=np.float32)
    mask = np.asarray(mask)

    in_maps = []
    for c in range(NC_CORES):
        b, g = divmod(c, 2)
        hs = slice(g * CW, (g + 1) * CW)
        mrow = (mask[b, 0, 0, :] != 0).astype(np.float32)
        in_maps.append({
            "xqT": np.ascontiguousarray(query[b].T).astype(bf),
            "xkT": np.ascontiguousarray(key[b].T).astype(bf),
            "xvT": np.ascontiguousarray(value[b].T).astype(bf),
            "wqT": np.ascontiguousarray(W_q[hs, :].T).astype(bf),
            "wkT": np.ascontiguousarray(W_k[hs, :].T).astype(bf),
            "wvT": np.ascontiguousarray(W_v[hs, :].T).astype(bf),
            "woT": np.ascontiguousarray(W_o[:, hs].T).astype(bf),
            "maskf": np.ascontiguousarray(mrow.reshape(KT, 128).T),
        })
    res = run_bass_kernel_spmd(_NC, in_maps, core_ids=list(range(NC_CORES)))
    out = np.empty((B, S, DM), np.float32)
    for b in range(B):
        out[b] = (res.results[2 * b]["outT"].astype(np.float32)
                  + res.results[2 * b + 1]["outT"].astype(np.float32)).T
    return out


# revision 6
# speedup vs baseline: 1.0466x; 1.0466x over previous
"""MultiHeadAttention Trainium2 kernel.

Sharding: 8 cores = 4 batches x 2 head-groups (8 heads each).
Each core computes, for its (batch b, head-group g):
  Q^T = Wq_g @ Xq^T, K^T = Wk_g @ Xk^T   (bf16 inputs/weights, f32 PSUM,
  [headdim, S] layout), V = Xv @ Wv_g^T  ([S, 512] layout, +ones col,
  mask-scaled, bf16), scores^T[k,q] per head (K=64 matmuls),
  e = exp(s/8) on ACT (PSUM->SBUF, f32r).
  attnV runs with e as the STATIONARY operand and V as the 65-wide moving
  operand (x~[q, hd] += e_chunk^T-weighted V), so the PE pays 65 cols
  instead of 512 per (head, k-tile, q-chunk): the softmax denominator is
  the ones column and lands per-PARTITION, so normalization is a plain
  per-partition reciprocal + scalar multiply on DVE.  The normalized
  x~[q, hd] staging tile is transposed back to [hd, q] layout with a
  cheap DMA xbar transpose (SP/HWDGE/DMA engines, zero PE cost), then
  out^T_partial = Wo_g^T.T @ x^T (bf16).
Host sums the two head-group partials per batch and transposes back.

Mask handling: V rows and the ones column are multiplied by mask (0/1), which
masks both the attnV numerator and the softmax denominator exactly.

DMA traffic runs in bf16 (inputs, weights, out partials) and is batched into
whole-block transfers (the descriptor engine costs ~625ns per DMA, so many
small DMAs serialize); PSUM accumulation stays f32 and exp output stays f32r.
"""
import contextlib
import os

import numpy as np
import ml_dtypes
import concourse.bass as bass  # noqa: F401
import concourse.tile as tile
from concourse import bacc, mybir
from concourse.bass_utils import run_bass_kernel_spmd

F32 = mybir.dt.float32
F32R = mybir.dt.float32r
BF16 = mybir.dt.bfloat16
EXP = mybir.ActivationFunctionType.Exp

B, S, DM = 4, 2048, 1024
H = 16
DK = 64
HLOC = 8              # heads per core
CW = HLOC * DK        # 512 local head dims per core
NC_CORES = 8
KT = S // 128         # 16 k-tiles
NB = S // 512         # 4 q/s blocks of 512
MT = CW // 128        # 4 m-tiles of local head dims
DT = DM // 128        # 8 contraction tiles over d_model
SCALE = 1.0 / np.sqrt(DK)

_NC = None


def _env(k, d):
    return int(os.environ.get(k, d))


def _build():
    nc = bacc.Bacc()
    xqT = nc.dram_tensor("xqT", [DM, S], BF16, kind="ExternalInput")
    xkT = nc.dram_tensor("xkT", [DM, S], BF16, kind="ExternalInput")
    xvT = nc.dram_tensor("xvT", [DM, S], BF16, kind="ExternalInput")
    wqT = nc.dram_tensor("wqT", [DM, CW], BF16, kind="ExternalInput")
    wkT = nc.dram_tensor("wkT", [DM, CW], BF16, kind="ExternalInput")
    wvT = nc.dram_tensor("wvT", [DM, CW], BF16, kind="ExternalInput")
    woT = nc.dram_tensor("woT", [CW, DM], BF16, kind="ExternalInput")
    maskf = nc.dram_tensor("maskf", [128, KT], F32, kind="ExternalInput")
    outT = nc.dram_tensor("outT", [DM, S], BF16, kind="ExternalOutput")

    # DRAM views with the k-tile dim split out: row (k*128+p) -> [p, k, cols]
    xqv = xqT.rearrange("(k p) s -> p k s", p=128)
    xkv = xkT.rearrange("(k p) s -> p k s", p=128)
    xvv = xvT.rearrange("(k p) s -> p k s", p=128)
    wqv = wqT.rearrange("(k p) c -> p k c", p=128)
    wkv = wkT.rearrange("(k p) c -> p k c", p=128)
    wvv = wvT.rearrange("(k p) c -> p k c", p=128)
    wov = woT.rearrange("(k p) c -> p k c", p=128)
    outv = outT.rearrange("(m p) s -> p m s", p=128)

    with tile.TileContext(nc) as tc, contextlib.ExitStack() as ctx:
        persist = ctx.enter_context(tc.tile_pool(name="persist", bufs=1))

        # --- persistent tiles: mask, wo, Q^T/K^T slices, V ---
        m_sb = persist.tile([128, KT], F32)
        nc.sync.dma_start(m_sb[:], maskf[:])
        ones8 = persist.tile([128, HLOC], F32)
        nc.vector.memset(ones8[:], 1.0)
        warm = persist.tile([1, 1], F32)
        nc.scalar.activation(warm[:], ones8[0:1, 0:1], EXP, scale=1.0)
        q_tiles = {}   # (m, nb) -> [128, 512] bf16  (Q^T slice)
        k_tiles = {}
        for m in range(MT):
            for n in range(NB):
                q_tiles[(m, n)] = persist.tile(
                    [128, 512], BF16, tag=f"q{m}_{n}", name=f"q{m}_{n}")
                k_tiles[(m, n)] = persist.tile(
                    [128, 512], BF16, tag=f"k{m}_{n}", name=f"k{m}_{n}")
        v_sb = persist.tile([128, KT, HLOC, DK + 1], BF16, tag="v")
        wo_t = persist.tile([128, MT, DM], BF16, tag="wo")

        # ---------------- Phase A: projections ----------------
        wq_pool = ctx.enter_context(tc.tile_pool(name="wqp", bufs=1))
        xt = ctx.enter_context(tc.tile_pool(name="xt", bufs=_env("K_XT_BUFS", 6)))
        ctxA = contextlib.ExitStack()
        with ctxA:
            wkv_pool = ctxA.enter_context(tc.tile_pool(name="wkv", bufs=1))
            psA = ctxA.enter_context(tc.tile_pool(name="psA", bufs=8, space="PSUM"))
            wq_sb = wq_pool.tile([128, DT, CW], BF16, tag="wq")
            wk_sb = wkv_pool.tile([128, DT, CW], BF16, tag="wk")
            wv_sb = wq_pool.tile([128, DT, CW], BF16, tag="wv")

            def dma_block(srcv, n, nm, halves=False):
                """One batched DMA (or two halves) for an x block: returns
                [128, DT, 512] bf16 tile."""
                xts = xt.tile([128, DT, 512], BF16, tag="xt", name=f"{nm}{n}")
                cs = slice(n * 512, (n + 1) * 512)
                if halves:
                    h = DT // 2
                    nc.sync.dma_start(xts[:, 0:h, :], srcv[:, 0:h, cs])
                    nc.sync.dma_start(xts[:, h:DT, :], srcv[:, h:DT, cs])
                else:
                    nc.sync.dma_start(xts[:], srcv[:, :, cs])
                return xts

            # k-major projection block: 4 PSUM groups accumulate in lockstep
            # so the first matmul only waits on the first half-DMAs.
            def proj_block_kmajor(dst_tiles, w_sb, xts, n, nm,
                                  split_evac=False, mlist=None):
                mlist = list(range(MT)) if mlist is None else mlist
                ps = {m: psA.tile([128, 512], F32, tag="pa",
                                  name=f"pj{nm}{n}_{m}") for m in mlist}
                for k in range(DT):
                    for m in mlist:
                        nc.tensor.matmul(
                            ps[m][:], w_sb[:, k, m * 128:(m + 1) * 128],
                            xts[:, k, :], start=(k == 0), stop=(k == DT - 1))
                for m in mlist:
                    if split_evac and m % 2:
                        nc.scalar.copy(dst_tiles[(m, n)][:], ps[m][:])
                    else:
                        nc.vector.tensor_copy(dst_tiles[(m, n)][:], ps[m][:])

            # single projection group (phase-B side work; DMAs long done)
            def proj_group(dst_tiles, w_sb, xts, n, m, pool, tag):
                ps = pool.tile([128, 512], F32, tag=tag, name=f"pj{n}_{m}_{tag}")
                for k in range(DT):
                    nc.tensor.matmul(
                        ps[:], w_sb[:, k, m * 128:(m + 1) * 128],
                        xts[:, k, :], start=(k == 0), stop=(k == DT - 1))
                nc.vector.tensor_copy(dst_tiles[(m, n)][:], ps[:])

            def v_evac(n, sm, ps):
                t = n * 4 + sm
                nc.vector.tensor_scalar_mul(
                    v_sb[:, t, :, 0:DK],
                    ps[:].rearrange("p (h d) -> p h d", h=HLOC),
                    m_sb[:, t:t + 1])
                nc.vector.tensor_scalar_mul(
                    v_sb[:, t, :, DK:DK + 1], ones8[:],
                    m_sb[:, t:t + 1])

            def v_block_kmajor(n, xts):
                ps = [psA.tile([128, 512], F32, tag="pa",
                               name=f"vps{n}_{sm}") for sm in range(4)]
                for k in range(DT):
                    for sm in range(4):
                        nc.tensor.matmul(
                            ps[sm][:], xts[:, k, sm * 128:(sm + 1) * 128],
                            wv_sb[:, k, :], start=(k == 0), stop=(k == DT - 1))
                for sm in range(4):
                    v_evac(n, sm, ps[sm])

            def v_group(n, sm, xts, pool, tag):
                ps = pool.tile([128, 512], F32, tag=tag, name=f"vps{n}_{sm}")
                for k in range(DT):
                    nc.tensor.matmul(
                        ps[:], xts[:, k, sm * 128:(sm + 1) * 128],
                        wv_sb[:, k, :], start=(k == 0), stop=(k == DT - 1))
                v_evac(n, sm, ps)

            # Phase-A DMA issue order = consumption order.
            hh = DT // 2
            qq = DT // 4
            nc.sync.dma_start(wk_sb[:, 0:qq, :], wkv[:, 0:qq, :])
            xk0 = xt.tile([128, DT, 512], BF16, tag="xt", name="xk0")
            nc.sync.dma_start(xk0[:, 0:qq, :], xkv[:, 0:qq, 0:512])
            nc.sync.dma_start(wk_sb[:, qq:hh, :], wkv[:, qq:hh, :])
            nc.sync.dma_start(xk0[:, qq:hh, :], xkv[:, qq:hh, 0:512])
            nc.sync.dma_start(wk_sb[:, hh:DT, :], wkv[:, hh:DT, :])
            nc.sync.dma_start(xk0[:, hh:DT, :], xkv[:, hh:DT, 0:512])
            xk_blocks = [xk0] + [dma_block(xkv, n, "xk") for n in range(1, NB)]
            nc.sync.dma_start(wq_sb[:], wqv[:])
            xq0 = dma_block(xqv, 0, "xq")
            nc.sync.dma_start(wv_sb[:], wvv[:])
            xv0 = dma_block(xvv, 0, "xv")
            xv1 = dma_block(xvv, 1, "xv")
            nc.sync.dma_start(wo_t[:], wov[:])

            # PE warmup: dummy matmuls cover initial DMA latency and start
            # the HAM activity window before the first real matmul.
            dum = wq_pool.tile([128, 512], BF16, tag="dum")
            nc.gpsimd.memset(dum[:], 0.0)
            for i in range(_env("K_WARM_MM", 2)):
                pw = psA.tile([128, 512], F32, tag="pa", name=f"warmmm{i}")
                for rep in range(_env("K_WARM_REP", 5)):
                    nc.tensor.matmul(pw[:], dum[:, 0:128], dum[:],
                                     start=(rep == 0), stop=True)
            for n in range(NB):
                proj_block_kmajor(k_tiles, wk_sb, xk_blocks[n], n, "xk")
            v_block_kmajor(0, xv0)
            v_block_kmajor(1, xv1)
            xv2 = dma_block(xvv, 2, "xv")
            xv3 = dma_block(xvv, 3, "xv")
            proj_block_kmajor(q_tiles, wq_sb, xq0, 0, "xq",
                              split_evac=True, mlist=[0, 1])

        # ---------------- Phase B: attention + out-proj ----------------
        # q blocks: three 512-wide (SGW=2), two 256-wide (SGW=4) so the
        # serial final out-projection tail is halved. Narrow blocks keep the
        # exp instruction count low by covering 4 k-tiles per activation.
        QB = [(0, 512, 2), (512, 512, 2), (1024, 512, 2),
              (1536, 256, 4), (1792, 256, 4)]
        NQB = len(QB)
        with tc.tile_pool(name="ev", bufs=_env("K_EV_BUFS", 4)) as ev, \
             tc.tile_pool(name="x", bufs=2) as xpool, \
             tc.tile_pool(name="xn", bufs=2) as xnpool, \
             tc.tile_pool(name="small", bufs=_env("K_SMALL_BUFS", 4)) as small, \
             tc.tile_pool(name="o", bufs=2) as opool, \
             tc.tile_pool(name="psS", bufs=_env("K_PSS_BUFS", 3), space="PSUM") as psS, \
             tc.tile_pool(name="psX", bufs=_env("K_XO_BUFS", 2), space="PSUM") as psX:
            x_tiles = [xpool.tile([128, MT, 512], BF16, tag="xs",
                                  name=f"xs{i}") for i in range(2)]
            xn_stage = [xnpool.tile([128, 4, 512], BF16, tag="xn",
                                    name=f"xn{i}") for i in range(2)]
            o_tiles = [opool.tile([128, DT, 512], BF16, tag="ob",
                                  name=f"ob{i}") for i in range(2)]

            def outproj_group(oqb, m, flush=False):
                col0, W, _ = QB[oqb]
                x_prev = x_tiles[oqb % 2]
                o_sb = o_tiles[oqb % 2]
                po = psS.tile([128, W], F32, tag="s", name=f"po{oqb}_{m}")
                for kk in range(MT):
                    nc.tensor.matmul(
                        po[:], wo_t[:, kk, m * 128:(m + 1) * 128],
                        x_prev[:, kk, 0:W], start=(kk == 0), stop=(kk == MT - 1))
                nc.vector.tensor_copy(o_sb[:, m, 0:W], po[:])
                if flush:
                    # batched output DMA for this q block
                    nc.sync.dma_start(
                        outv[:, :, col0:col0 + W], o_sb[:, :, 0:W])

            # side-work: one psS-slot matmul group (or a DMA batch) per sg
            # step. v-block deadline: attnV eats V tile t at emission slot
            # t//SGW+1. Q_n must be complete before q block n starts.
            xts_store = {("v", 2): xv2, ("v", 3): xv3, ("q", 0): xq0}

            def mk_vg(nn, sm):
                return ("mm", lambda: v_group(nn, sm, xts_store[("v", nn)],
                                              psS, "s"))

            def mk_qdma(nn):
                def f():
                    xts_store[("q", nn)] = dma_block(xqv, nn, "xq")
                return ("dma", f)

            def mk_qg(nn, m):
                return ("mm", lambda: proj_group(q_tiles, wq_sb,
                                                 xts_store[("q", nn)],
                                                 nn, m, psS, "s"))

            def mk_og(oqb, m, flush=False):
                return ("mm", lambda: outproj_group(oqb, m, flush))

            # (qb, p) -> [(min_sg, (kind, fn)), ...]
            side_work = {}
            VOFF = _env("K_VOFF", 1)
            side_work[(0, 0)] = [
                (max(0, VOFF + i), mk_vg(2 + i // 4, i % 4)) for i in range(8)]
            # Per-pair balancing: every pair (not just p0) hosts enough side
            # matmul groups to keep PE ahead of the ACT exp stream. Q_n's
            # m-groups spread across the hosting block's pairs (group m is
            # only needed when block n reaches pair m). og of block i may
            # only run while x_tiles[i%2] is intact: anywhere in block i+1,
            # but only in block i+2's p0 early slots.
            side_work[(0, 1)] = [(0, mk_qdma(1)), (3, mk_qg(0, 2)),
                                 (6, mk_qg(1, 0))]
            side_work[(0, 2)] = [(3, mk_qg(0, 3)), (6, mk_qg(1, 1))]
            side_work[(0, 3)] = [(3, mk_qg(1, 2)), (6, mk_qg(1, 3))]
            SIDE = {
                (1, 0): [(0, 'qdma', 2), (2, 'og', 0, 0), (7, 'og', 0, 1),
                         (4, 'qg', 2, 0)],
                (1, 1): [(2, 'og', 0, 2), (7, 'og', 0, 3), (4, 'qg', 2, 1)],
                (1, 2): [(2, 'og', 0, 4), (7, 'og', 0, 5), (4, 'qg', 2, 2)],
                (1, 3): [(2, 'og', 0, 6), (7, 'og', 0, 7), (4, 'qg', 2, 3)],
                (2, 0): [(0, 'qdma', 3), (2, 'og', 1, 0), (7, 'og', 1, 1),
                         (4, 'qg', 3, 0)],
                (2, 1): [(2, 'og', 1, 2), (7, 'og', 1, 3), (4, 'qg', 3, 1)],
                (2, 2): [(2, 'og', 1, 4), (7, 'og', 1, 5), (4, 'qg', 3, 2)],
                (2, 3): [(2, 'og', 1, 6), (7, 'og', 1, 7), (4, 'qg', 3, 3)],
                (3, 0): [(1, 'og', 2, 0)],
                (3, 1): [(1, 'og', 2, 2), (2, 'og', 2, 3)],
                (3, 2): [(1, 'og', 2, 4), (3, 'og', 2, 1)],
                (3, 3): [(1, 'og', 2, 5)],
                (4, 0): [(0, 'og', 2, 6), (2, 'og', 2, 7)],
                (4, 1): [(0, 'og', 3, 0), (1, 'og', 3, 1), (3, 'og', 3, 2)],
                (4, 2): [(0, 'og', 3, 3), (1, 'og', 3, 4), (3, 'og', 3, 5)],
                (4, 3): [(0, 'og', 3, 6), (1, 'og', 3, 7)],
            }
            for key, items in SIDE.items():
                lst = side_work.setdefault(key, [])
                for it in items:
                    if it[1] == 'qdma':
                        lst.append((it[0], mk_qdma(it[2])))
                    elif it[1] == 'qg':
                        lst.append((it[0], mk_qg(it[2], it[3])))
                    else:
                        lst.append((it[0], mk_og(it[2], it[3],
                                                 flush=(it[3] == DT - 1))))
            for key in side_work:
                side_work[key].sort(key=lambda it: it[0])

            MAXMM = _env("K_MAXMM", 1)

            def side_step(qb, p, sg):
                work = side_work.get((qb, p))
                if not work:
                    return
                did_mm = 0
                while work:
                    min_sg, (kind, fn) = work[0]
                    if min_sg > sg or (kind == "mm" and did_mm >= MAXMM):
                        break
                    work.pop(0)
                    fn()
                    if kind == "mm":
                        did_mm += 1

            def side_flush(qb, p):
                for _, (kind, fn) in side_work.pop((qb, p), []):
                    fn()

            def attn_v(ps_x, h, sg, sgw, e_h, nqc):
                """Flipped attnV for supergroup sg: e chunks stationary,
                V [128, 65] moving, accumulating x~[q, hd|den] per qc."""
                for tt in range(sgw):
                    t = sg * sgw + tt
                    for qc in range(nqc):
                        # start=True zeroes the WHOLE psum bank, so only the
                        # very first matmul into this tile may set it; the
                        # other qc groups accumulate onto the zeroed bank.
                        nc.tensor.matmul(
                            ps_x[h][:, qc, :],
                            e_h[:, tt, qc * 128:(qc + 1) * 128],
                            v_sb[:, t, h, :],
                            start=(t == 0 and qc == 0), stop=(t == KT - 1))

            for qb in range(NQB):
                col0, W, sgw = QB[qb]
                nb = col0 // 512
                q0 = col0 % 512
                nsg = KT // sgw
                nqc = W // 128
                xn_sb = xn_stage[qb % 2]
                for p in range(MT):        # head pairs; pair p = heads 2p,2p+1
                    heads = (2 * p, 2 * p + 1)
                    ps_x = {h: psX.tile([128, nqc, DK + 1], F32, tag="xo",
                                        name=f"psx{qb}_{h}") for h in heads}
                    e_prev = None
                    for sg in range(nsg):
                        # side work: outproj of qb-1, V, or late q projection
                        side_step(qb, p, sg)
                        e_new = {}
                        for h in heads:
                            hp = h % 2
                            ps_h = psS.tile([128, sgw, W], F32, tag="s",
                                            name=f"pss{qb}_{sg}_{h}")
                            for tt in range(sgw):
                                t = sg * sgw + tt
                                nc.tensor.matmul(
                                    ps_h[:, tt, :],
                                    k_tiles[(p, t // 4)][
                                        hp * 64:(hp + 1) * 64,
                                        (t % 4) * 128:(t % 4 + 1) * 128],
                                    q_tiles[(p, nb)][hp * 64:(hp + 1) * 64,
                                                     q0:q0 + W],
                                    start=True, stop=True)
                            # hand this head's exp to ACT immediately, then
                            # run its previous-supergroup attnV (1-sg lag)
                            # while the other head's scores stream
                            e_sb = ev.tile([128, sgw, W], BF16, tag="e",
                                           name=f"e{qb}_{sg}_{h}")
                            nc.scalar.activation(e_sb[:], ps_h[:], EXP,
                                                 scale=float(SCALE))
                            e_new[h] = e_sb
                            if e_prev is not None:
                                attn_v(ps_x, h, sg - 1, sgw, e_prev[h], nqc)
                        e_prev = e_new
                    side_flush(qb, p)
                    # drain last supergroup + normalize into xn staging
                    for h in heads:
                        attn_v(ps_x, h, nsg - 1, sgw, e_prev[h], nqc)
                        hp = h % 2
                        c0 = p * 128 + hp * 64
                        for qc in range(nqc):
                            r = small.tile([128, 1], F32, tag="r",
                                           name=f"r{qb}_{h}_{qc}")
                            nc.vector.reciprocal(r[:], ps_x[h][:, qc, DK:DK + 1])
                            nc.vector.tensor_scalar_mul(
                                xn_sb[:, qc, c0:c0 + 64],
                                ps_x[h][:, qc, 0:DK], r[:])
                    # x~ staged as [q, hd]; flip this pair's slice back to
                    # [hd, q] via the DMA xbar transpose (SP queue + DMA
                    # engines, no PE cost). Per-pair so the next block's
                    # outproj kk-matmuls find their deps already satisfied.
                    for qc in range(nqc):
                        nc.sync.dma_start_transpose(
                            x_tiles[qb % 2][:, p, qc * 128:(qc + 1) * 128],
                            xn_sb[:, qc, p * 128:(p + 1) * 128])
            # final out-projection for the last q block (its og side-work
            # can't ride a following block).
            oqb = NQB - 1
            col0, W, _ = QB[oqb]
            o_sb = o_tiles[oqb % 2]
            x_prev = x_tiles[oqb % 2]
            for m in range(DT):
                ms = slice(m * 128, (m + 1) * 128)
                po = psS.tile([128, W], F32, tag="s", name=f"pof{m}")
                for kk in range(MT):
                    nc.tensor.matmul(
                        po[:], wo_t[:, kk, ms], x_prev[:, kk, 0:W],
                        start=(kk == 0), stop=(kk == MT - 1))
                if m % 2:
                    nc.scalar.copy(o_sb[:, m, 0:W], po[:])
                else:
                    nc.vector.tensor_copy(o_sb[:, m, 0:W], po[:])
                if m == 3:
                    nc.sync.dma_start(
                        outv[:, 0:4, col0:col0 + W], o_sb[:, 0:4, 0:W])
                elif m == 6:
                    nc.sync.dma_start(
                        outv[:, 4:7, col0:col0 + W], o_sb[:, 4:7, 0:W])
            nc.sync.dma_start(
                outv[:, 7:8, col0:col0 + W], o_sb[:, 7:8, 0:W])
    nc.finalize()
    return nc


def kernel(query, key, value, mask, W_q, W_k, W_v, W_o):
    global _NC
    if _NC is None:
        _NC = _build()
    bf = ml_dtypes.bfloat16
    query = np.asarray(query, dtype=np.float32)
    key = np.asarray(key, dtype=np.float32)
    value = np.asarray(value, dtype=np.float32)
    W_q = np.asarray(W_q, dtype=np.float32)
    W_k = np.asarray(W_k, dtype=np.float32)
    W_v = np.asarray(W_v, dtype=np.float32)
    W_o = np.asarray(W_o, dtype=np.float32)
    mask = np.asarray(mask)

    in_maps = []
    for c in range(NC_CORES):
        b, g = divmod(c, 2)
        hs = slice(g * CW, (g + 1) * CW)
        mrow = (mask[b, 0, 0, :] != 0).astype(np.float32)
        in_maps.append({
            "xqT": np.ascontiguousarray(query[b].T).astype(bf),
            "xkT": np.ascontiguousarray(key[b].T).astype(bf),
            "xvT": np.ascontiguousarray(value[b].T).astype(bf),
            "wqT": np.ascontiguousarray(W_q[hs, :].T).astype(bf),
            "wkT": np.ascontiguousarray(W_k[hs, :].T).astype(bf),
            "wvT": np.ascontiguousarray(W_v[hs, :].T).astype(bf),
            "woT": np.ascontiguousarray(W_o[:, hs].T).astype(bf),
            "maskf": np.ascontiguousarray(mrow.reshape(KT, 128).T),
        })
    res = run_bass_kernel_spmd(_NC, in_maps, core_ids=list(range(NC_CORES)))
    out = np.empty((B, S, DM), np.float32)
    for b in range(B):
        out[b] = (res.results[2 * b]["outT"].astype(np.float32)
                  + res.results[2 * b + 1]["outT"].astype(np.float32)).T
    return out


# revision 7
# speedup vs baseline: 1.0527x; 1.0059x over previous
"""MultiHeadAttention Trainium2 kernel.

Sharding: 8 cores = 4 batches x 2 head-groups (8 heads each).
Each core computes, for its (batch b, head-group g):
  Q^T = Wq_g @ Xq^T, K^T = Wk_g @ Xk^T   (bf16 inputs/weights, f32 PSUM,
  [headdim, S] layout), V = Xv @ Wv_g^T  ([S, 512] layout, +ones col,
  mask-scaled, bf16), scores^T[k,q] per head (K=64 matmuls),
  e = exp(s/8) on ACT (PSUM->SBUF, bf16).
  attnV runs with e as the STATIONARY operand and V as the 65-wide moving
  operand (x~[q, hd] += e_chunk^T-weighted V), so the PE pays 65 cols
  instead of 512 per (head, k-tile, q-chunk): the softmax denominator is
  the ones column and lands per-PARTITION, so normalization is a plain
  per-partition reciprocal + scalar multiply on DVE.  The normalized
  x~[q, hd] staging tile is transposed back to [hd, q] layout with a
  cheap DMA xbar transpose (SP/HWDGE/DMA engines, zero PE cost), then
  out^T_partial = Wo_g^T.T @ x^T (bf16).
Host sums the two head-group partials per batch and transposes back.

Scheduling: the serial ramp is minimal (K m0 block0 + Q m0 block0 only,
~11us to the first exp); everything else (remaining K blocks/m-tiles,
per-head-pair V projection units, Q m-tiles, out-projections of the
previous q block) runs as deadline-scheduled side work inside the
phase-B supergroup loop, keeping ACT (the exp stream, the long pole)
fed as early and as continuously as possible.  attnV lags the exp
stream by 2 supergroups so V-projection side units have time to land.

Mask handling: V rows and the ones column are multiplied by mask (0/1), which
masks both the attnV numerator and the softmax denominator exactly.
"""
import contextlib
import os

import numpy as np
import ml_dtypes
import concourse.bass as bass  # noqa: F401
import concourse.tile as tile
from concourse import bacc, mybir
from concourse.bass_utils import run_bass_kernel_spmd

F32 = mybir.dt.float32
F32R = mybir.dt.float32r
BF16 = mybir.dt.bfloat16
EXP = mybir.ActivationFunctionType.Exp

B, S, DM = 4, 2048, 1024
H = 16
DK = 64
HLOC = 8              # heads per core
CW = HLOC * DK        # 512 local head dims per core
NC_CORES = 8
KT = S // 128         # 16 k-tiles
NB = S // 512         # 4 q/s blocks of 512
MT = CW // 128        # 4 m-tiles of local head dims
DT = DM // 128        # 8 contraction tiles over d_model
SCALE = 1.0 / np.sqrt(DK)
LAG = 2               # attnV supergroup lag behind the exp stream

_NC = None


def _env(k, d):
    return int(os.environ.get(k, d))


def _build():
    nc = bacc.Bacc()
    xqT = nc.dram_tensor("xqT", [DM, S], BF16, kind="ExternalInput")
    xkT = nc.dram_tensor("xkT", [DM, S], BF16, kind="ExternalInput")
    xvT = nc.dram_tensor("xvT", [DM, S], BF16, kind="ExternalInput")
    wqT = nc.dram_tensor("wqT", [DM, CW], BF16, kind="ExternalInput")
    wkT = nc.dram_tensor("wkT", [DM, CW], BF16, kind="ExternalInput")
    wvT = nc.dram_tensor("wvT", [DM, CW], BF16, kind="ExternalInput")
    woT = nc.dram_tensor("woT", [CW, DM], BF16, kind="ExternalInput")
    maskf = nc.dram_tensor("maskf", [128, KT], F32, kind="ExternalInput")
    outT = nc.dram_tensor("outT", [DM, S], BF16, kind="ExternalOutput")

    # DRAM views with the k-tile dim split out: row (k*128+p) -> [p, k, cols]
    xqv = xqT.rearrange("(k p) s -> p k s", p=128)
    xkv = xkT.rearrange("(k p) s -> p k s", p=128)
    xvv = xvT.rearrange("(k p) s -> p k s", p=128)
    wqv = wqT.rearrange("(k p) c -> p k c", p=128)
    wkv = wkT.rearrange("(k p) c -> p k c", p=128)
    wvv = wvT.rearrange("(k p) c -> p k c", p=128)
    wov = woT.rearrange("(k p) c -> p k c", p=128)
    outv = outT.rearrange("(m p) s -> p m s", p=128)

    with tile.TileContext(nc) as tc, contextlib.ExitStack() as ctx:
        persist = ctx.enter_context(tc.tile_pool(name="persist", bufs=1))

        # --- persistent tiles: mask, wo, Q^T/K^T slices, V ---
        m_sb = persist.tile([128, KT], F32)
        nc.sync.dma_start(m_sb[:], maskf[:])
        ones8 = persist.tile([128, HLOC], F32)
        nc.vector.memset(ones8[:], 1.0)
        warm = persist.tile([1, 1], F32)
        nc.scalar.activation(warm[:], ones8[0:1, 0:1], EXP, scale=1.0)
        q_tiles = {}   # (m, nb) -> [128, 512] bf16  (Q^T slice)
        k_tiles = {}
        for m in range(MT):
            for n in range(NB):
                q_tiles[(m, n)] = persist.tile(
                    [128, 512], BF16, tag=f"q{m}_{n}", name=f"q{m}_{n}")
                k_tiles[(m, n)] = persist.tile(
                    [128, 512], BF16, tag=f"k{m}_{n}", name=f"k{m}_{n}")
        v_sb = persist.tile([128, KT, HLOC, DK + 1], BF16, tag="v")
        wo_t = persist.tile([128, MT, DM], BF16, tag="wo")

        # weights persist through phase B (K/V/Q side units use them)
        wq_pool = ctx.enter_context(tc.tile_pool(name="wqp", bufs=1))
        xt = ctx.enter_context(tc.tile_pool(name="xt", bufs=_env("K_XT_BUFS", 10)))
        wq_sb = wq_pool.tile([128, DT, CW], BF16, tag="wq")
        wk_sb = wq_pool.tile([128, DT, CW], BF16, tag="wk")
        wv_sb = wq_pool.tile([128, DT, CW], BF16, tag="wv")
        dum = wq_pool.tile([128, 512], BF16, tag="dum")

        def dma_block(srcv, n, nm):
            """One batched DMA for an x block: [128, DT, 512] bf16 tile."""
            xts = xt.tile([128, DT, 512], BF16, tag="xt", name=f"{nm}{n}")
            nc.sync.dma_start(xts[:], srcv[:, :, n * 512:(n + 1) * 512])
            return xts

        # single projection m-group: 8 accumulating matmuls + DVE evac
        def proj_group(dst_tiles, w_sb, xts, n, m, pool, tag):
            ps = pool.tile([128, 512], F32, tag=tag, name=f"pj{n}_{m}_{tag}")
            for k in range(DT):
                nc.tensor.matmul(
                    ps[:], w_sb[:, k, m * 128:(m + 1) * 128],
                    xts[:, k, :], start=(k == 0), stop=(k == DT - 1))
            nc.vector.tensor_copy(dst_tiles[(m, n)][:], ps[:])

        def v_group_pair(n, sm, p, pool, tag):
            """V projection for k-tile t=n*4+sm, head pair p only (128 cols):
            V[kpos, 2 heads x 64] + mask scaling into v_sb."""
            t = n * 4 + sm
            ps = pool.tile([128, 128], F32, tag=tag, name=f"vp{t}_{p}")
            for k in range(DT):
                nc.tensor.matmul(
                    ps[:], xts_store[("v", n)][:, k, sm * 128:(sm + 1) * 128],
                    wv_sb[:, k, p * 128:(p + 1) * 128],
                    start=(k == 0), stop=(k == DT - 1))
            nc.vector.tensor_scalar_mul(
                v_sb[:, t, 2 * p:2 * p + 2, 0:DK],
                ps[:].rearrange("p (h d) -> p h d", h=2),
                m_sb[:, t:t + 1])
            nc.vector.tensor_scalar_mul(
                v_sb[:, t, 2 * p:2 * p + 2, DK:DK + 1], ones8[:, 0:2],
                m_sb[:, t:t + 1])

        # ---------------- Phase A: minimal serial ramp ----------------
        # DMA issue order = consumption order; the DMA engine pool is a
        # serial resource so order is everything.  PE warmup covers the
        # first DMAs and starts the pstate ramp.
        ctxA = contextlib.ExitStack()
        with ctxA:
            psA = ctxA.enter_context(tc.tile_pool(name="psA", bufs=4, space="PSUM"))
            nc.sync.dma_start(wk_sb[:, :, 0:128], wkv[:, :, 0:128])
            xk0 = dma_block(xkv, 0, "xk")
            nc.sync.dma_start(wq_sb[:, :, 0:128], wqv[:, :, 0:128])
            xq0 = dma_block(xqv, 0, "xq")
            nc.sync.dma_start(wv_sb[:], wvv[:])
            xv_blocks = [dma_block(xvv, 0, "xv")]
            xk_blocks = [xk0, dma_block(xkv, 1, "xk")]
            xv_blocks.append(dma_block(xvv, 1, "xv"))
            xk_blocks.append(dma_block(xkv, 2, "xk"))
            xk_blocks.append(dma_block(xkv, 3, "xk"))
            xv_blocks.append(dma_block(xvv, 2, "xv"))
            xv_blocks.append(dma_block(xvv, 3, "xv"))
            nc.sync.dma_start(wk_sb[:, :, 128:CW], wkv[:, :, 128:CW])
            nc.sync.dma_start(wq_sb[:, :, 128:CW], wqv[:, :, 128:CW])
            nc.sync.dma_start(wo_t[:], wov[:])

            nc.gpsimd.memset(dum[:], 0.0)
            for i in range(_env("K_WARM_MM", 2)):
                pw = psA.tile([128, 512], F32, tag="pa", name=f"warmmm{i}")
                for rep in range(_env("K_WARM_REP", 5)):
                    nc.tensor.matmul(pw[:], dum[:, 0:128], dum[:],
                                     start=(rep == 0), stop=True)
            proj_group(k_tiles, wk_sb, xk0, 0, 0, psA, "pa")
            proj_group(q_tiles, wq_sb, xq0, 0, 0, psA, "pa")

        # ---------------- Phase B: attention + out-proj ----------------
        QB = [(0, 512, 2), (512, 512, 2), (1024, 512, 2),
              (1536, 256, 4), (1792, 256, 4)]
        NQB = len(QB)
        with tc.tile_pool(name="ev", bufs=_env("K_EV_BUFS", 2 * (LAG + 1))) as ev, \
             tc.tile_pool(name="x", bufs=2) as xpool, \
             tc.tile_pool(name="xn", bufs=_env("K_XN_BUFS", 2)) as xnpool, \
             tc.tile_pool(name="small", bufs=_env("K_SMALL_BUFS", 4)) as small, \
             tc.tile_pool(name="o", bufs=2) as opool, \
             tc.tile_pool(name="psS", bufs=_env("K_PSS_BUFS", 3), space="PSUM") as psS, \
             tc.tile_pool(name="psX", bufs=_env("K_XO_BUFS", 2), space="PSUM") as psX:
            x_tiles = [xpool.tile([128, MT, 512], BF16, tag="xs",
                                  name=f"xs{i}") for i in range(2)]
            xn_stage = [xnpool.tile([128, 4, 512], BF16, tag="xn",
                                    name=f"xn{i}")
                        for i in range(_env("K_XN_BUFS", 2))]
            o_tiles = [opool.tile([128, DT, 512], BF16, tag="ob",
                                  name=f"ob{i}") for i in range(2)]

            def outproj_group(oqb, m, flush=False):
                col0, W, _ = QB[oqb]
                x_prev = x_tiles[oqb % 2]
                o_sb = o_tiles[oqb % 2]
                po = psS.tile([128, W], F32, tag="s", name=f"po{oqb}_{m}")
                for kk in range(MT):
                    nc.tensor.matmul(
                        po[:], wo_t[:, kk, m * 128:(m + 1) * 128],
                        x_prev[:, kk, 0:W], start=(kk == 0), stop=(kk == MT - 1))
                nc.vector.tensor_copy(o_sb[:, m, 0:W], po[:])
                if flush:
                    nc.sync.dma_start(
                        outv[:, :, col0:col0 + W], o_sb[:, :, 0:W])

            xts_store = {("v", n): xv_blocks[n] for n in range(NB)}
            xts_store[("q", 0)] = xq0

            def mk_vp(nn, sm, p):
                return ("mm", lambda: v_group_pair(nn, sm, p, psS, "s"))

            def mk_kg(m, b):
                return ("mm", lambda: proj_group(k_tiles, wk_sb, xk_blocks[b],
                                                 b, m, psS, "s"))

            def mk_qdma(nn):
                def f():
                    xts_store[("q", nn)] = dma_block(xqv, nn, "xq")
                return ("dma", f)

            def mk_qg(nn, m):
                return ("mm", lambda: proj_group(q_tiles, wq_sb,
                                                 xts_store[("q", nn)],
                                                 nn, m, psS, "s"))

            def mk_og(oqb, m, flush=False):
                return ("mm", lambda: outproj_group(oqb, m, flush))

            # (qb, p) -> [(min_sg, (kind, fn)), ...]
            # Block 0 hosts all remaining K m-tiles, per-pair V units and Q0
            # m-tiles, deadline-ordered: scores(p, sg) needs K m_p b(sg//2);
            # attnV at sg eats V t=sgw*(sg-LAG); pair p+1 needs K m_{p+1} b0
            # and Q0 m_{p+1} before it starts.
            side_work = {}

            def vp_sched(p, host_pair):
                """V units for pair p spread over hosting pair's sgs."""
                out = []
                for t in range(KT):
                    if host_pair == p:      # own pair: stay LAG sgs ahead
                        ms = max(1, t // 2)
                    else:                   # previous pair hosts: spread
                        ms = min(7, t // 2)
                    out.append((ms, mk_vp(t // 4, t % 4, p)))
                return out

            side_work[(0, 0)] = ([(1, mk_kg(0, 1)), (3, mk_kg(0, 2)),
                                  (5, mk_kg(0, 3)), (7, mk_kg(1, 0)),
                                  (7, mk_qg(0, 1))]
                                 + vp_sched(0, 0))
            side_work[(0, 1)] = ([(1, mk_kg(1, 1)), (3, mk_kg(1, 2)),
                                  (5, mk_kg(1, 3)), (7, mk_kg(2, 0)),
                                  (7, mk_qg(0, 2))]
                                 + vp_sched(1, 1))
            side_work[(0, 2)] = ([(2, mk_qdma(1)), (1, mk_kg(2, 1)),
                                  (3, mk_kg(2, 2)), (5, mk_kg(2, 3)),
                                  (7, mk_kg(3, 0)), (7, mk_qg(0, 3))]
                                 + vp_sched(2, 2))
            side_work[(0, 3)] = ([(1, mk_kg(3, 1)), (3, mk_kg(3, 2)),
                                  (5, mk_kg(3, 3)), (7, mk_qg(1, 0))]
                                 + vp_sched(3, 3))
            SIDE = {
                (1, 0): [(0, 'qdma', 2), (1, 'qg', 1, 1), (2, 'og', 0, 0),
                         (7, 'og', 0, 1), (4, 'qg', 2, 0)],
                (1, 1): [(0, 'qg', 1, 2), (2, 'og', 0, 2), (7, 'og', 0, 3),
                         (4, 'qg', 2, 1)],
                (1, 2): [(0, 'qg', 1, 3), (2, 'og', 0, 4), (7, 'og', 0, 5),
                         (4, 'qg', 2, 2)],
                (1, 3): [(2, 'og', 0, 6), (7, 'og', 0, 7), (4, 'qg', 2, 3)],
                (2, 0): [(0, 'qdma', 3), (2, 'og', 1, 0), (7, 'og', 1, 1),
                         (4, 'qg', 3, 0)],
                (2, 1): [(2, 'og', 1, 2), (7, 'og', 1, 3), (4, 'qg', 3, 1)],
                (2, 2): [(2, 'og', 1, 4), (7, 'og', 1, 5), (4, 'qg', 3, 2)],
                (2, 3): [(2, 'og', 1, 6), (7, 'og', 1, 7), (4, 'qg', 3, 3)],
                (3, 0): [(1, 'og', 2, 0)],
                (3, 1): [(1, 'og', 2, 2), (2, 'og', 2, 3)],
                (3, 2): [(1, 'og', 2, 4), (3, 'og', 2, 1)],
                (3, 3): [(1, 'og', 2, 5)],
                (4, 0): [(0, 'og', 2, 6), (2, 'og', 2, 7)],
                (4, 1): [(0, 'og', 3, 0), (1, 'og', 3, 1), (3, 'og', 3, 2)],
                (4, 2): [(0, 'og', 3, 3), (1, 'og', 3, 4), (3, 'og', 3, 5)],
                (4, 3): [(0, 'og', 3, 6), (1, 'og', 3, 7)],
            }
            for key, items in SIDE.items():
                lst = side_work.setdefault(key, [])
                for it in items:
                    if it[1] == 'qdma':
                        lst.append((it[0], mk_qdma(it[2])))
                    elif it[1] == 'qg':
                        lst.append((it[0], mk_qg(it[2], it[3])))
                    else:
                        lst.append((it[0], mk_og(it[2], it[3],
                                                 flush=(it[3] == DT - 1))))
            for key in side_work:
                side_work[key].sort(key=lambda it: it[0])

            MAXMM = _env("K_MAXMM", 1)
            MAXMM0 = _env("K_MAXMM0", 3)

            def side_step(qb, p, sg):
                work = side_work.get((qb, p))
                if not work:
                    return
                lim = MAXMM0 if qb == 0 else MAXMM
                did_mm = 0
                while work:
                    min_sg, (kind, fn) = work[0]
                    if min_sg > sg or (kind == "mm" and did_mm >= lim):
                        break
                    work.pop(0)
                    fn()
                    if kind == "mm":
                        did_mm += 1

            def side_flush(qb, p):
                for _, (kind, fn) in side_work.pop((qb, p), []):
                    fn()

            def attn_v(ps_x, h, sg, sgw, e_h, nqc):
                """Flipped attnV for supergroup sg: e chunks stationary,
                V [128, 65] moving, accumulating x~[q, hd|den] per qc."""
                for tt in range(sgw):
                    t = sg * sgw + tt
                    for qc in range(nqc):
                        # start=True zeroes the WHOLE psum bank, so only the
                        # very first matmul into this tile may set it; the
                        # other qc groups accumulate onto the zeroed bank.
                        nc.tensor.matmul(
                            ps_x[h][:, qc, :],
                            e_h[:, tt, qc * 128:(qc + 1) * 128],
                            v_sb[:, t, h, :],
                            start=(t == 0 and qc == 0), stop=(t == KT - 1))

            for qb in range(NQB):
                col0, W, sgw = QB[qb]
                nb = col0 // 512
                q0 = col0 % 512
                nsg = KT // sgw
                nqc = W // 128
                xn_sb = xn_stage[qb % len(xn_stage)]
                for p in range(MT):        # head pairs; pair p = heads 2p,2p+1
                    heads = (2 * p, 2 * p + 1)
                    ps_x = {h: psX.tile([128, nqc, DK + 1], F32, tag="xo",
                                        name=f"psx{qb}_{h}") for h in heads}
                    e_hist = {}
                    for sg in range(nsg):
                        side_step(qb, p, sg)
                        for h in heads:
                            hp = h % 2
                            ps_h = psS.tile([128, sgw, W], F32, tag="s",
                                            name=f"pss{qb}_{sg}_{h}")
                            for tt in range(sgw):
                                t = sg * sgw + tt
                                nc.tensor.matmul(
                                    ps_h[:, tt, :],
                                    k_tiles[(p, t // 4)][
                                        hp * 64:(hp + 1) * 64,
                                        (t % 4) * 128:(t % 4 + 1) * 128],
                                    q_tiles[(p, nb)][hp * 64:(hp + 1) * 64,
                                                     q0:q0 + W],
                                    start=True, stop=True)
                            e_sb = ev.tile([128, sgw, W], BF16, tag="e",
                                           name=f"e{qb}_{sg}_{h}")
                            nc.scalar.activation(e_sb[:], ps_h[:], EXP,
                                                 scale=float(SCALE))
                            e_hist[(sg, h)] = e_sb
                            if sg >= LAG:
                                attn_v(ps_x, h, sg - LAG, sgw,
                                       e_hist.pop((sg - LAG, h)), nqc)
                    side_flush(qb, p)
                    # drain last LAG supergroups + normalize into xn staging
                    for h in heads:
                        for j in range(LAG, 0, -1):
                            attn_v(ps_x, h, nsg - j, sgw,
                                   e_hist.pop((nsg - j, h)), nqc)
                        hp = h % 2
                        c0 = p * 128 + hp * 64
                        for qc in range(nqc):
                            r = small.tile([128, 1], F32, tag="r",
                                           name=f"r{qb}_{h}_{qc}")
                            nc.vector.reciprocal(r[:], ps_x[h][:, qc, DK:DK + 1])
                            nc.vector.tensor_scalar_mul(
                                xn_sb[:, qc, c0:c0 + 64],
                                ps_x[h][:, qc, 0:DK], r[:])
                    # x~ staged as [q, hd]; flip this pair's slice back to
                    # [hd, q] via the DMA xbar transpose (SP queue + DMA
                    # engines, no PE cost). Per-pair so the next block's
                    # outproj kk-matmuls find their deps already satisfied.
                    for qc in range(nqc):
                        nc.sync.dma_start_transpose(
                            x_tiles[qb % 2][:, p, qc * 128:(qc + 1) * 128],
                            xn_sb[:, qc, p * 128:(p + 1) * 128])
            # final out-projection for the last q block (its og side-work
            # can't ride a following block).
            oqb = NQB - 1
            col0, W, _ = QB[oqb]
            o_sb = o_tiles[oqb % 2]
            x_prev = x_tiles[oqb % 2]
            for m in range(DT):
                ms = slice(m * 128, (m + 1) * 128)
                po = psS.tile([128, W], F32, tag="s", name=f"pof{m}")
                for kk in range(MT):
                    nc.tensor.matmul(
                        po[:], wo_t[:, kk, ms], x_prev[:, kk, 0:W],
                        start=(kk == 0), stop=(kk == MT - 1))
                if m % 2:
                    nc.scalar.copy(o_sb[:, m, 0:W], po[:])
                else:
                    nc.vector.tensor_copy(o_sb[:, m, 0:W], po[:])
                if m == 3:
                    nc.sync.dma_start(
                        outv[:, 0:4, col0:col0 + W], o_sb[:, 0:4, 0:W])
                elif m == 6:
                    nc.sync.dma_start(
                        outv[:, 4:7, col0:col0 + W], o_sb[:, 4:7, 0:W])
            nc.sync.dma_start(
                outv[:, 7:8, col0:col0 + W], o_sb[:, 7:8, 0:W])
    nc.finalize()
    return nc


def kernel(query, key, value, mask, W_q, W_k, W_v, W_o):
    global _NC
    if _NC is None:
        _NC = _build()
    bf = ml_dtypes.bfloat16
    query = np.asarray(query, dtype=np.float32)
    key = np.asarray(key, dtype=np.float32)
    value = np.asarray(value, dtype=np.float32)
    W_q = np.asarray(W_q, dtype=np.float32)
    W_k = np.asarray(W_k, dtype=np.float32)
    W_v = np.asarray(W_v, dtype=np.float32)
    W_o = np.asarray(W_o, dtype=np.float32)
    mask = np.asarray(mask)

    in_maps = []
    for c in range(NC_CORES):
        b, g = divmod(c, 2)
        hs = slice(g * CW, (g + 1) * CW)
        mrow = (mask[b, 0, 0, :] != 0).astype(np.float32)
        in_maps.append({
            "xqT": np.ascontiguousarray(query[b].T).astype(bf),
            "xkT": np.ascontiguousarray(key[b].T).astype(bf),
            "xvT": np.ascontiguousarray(value[b].T).astype(bf),
            "wqT": np.ascontiguousarray(W_q[hs, :].T).astype(bf),
            "wkT": np.ascontiguousarray(W_k[hs, :].T).astype(bf),
            "wvT": np.ascontiguousarray(W_v[hs, :].T).astype(bf),
            "woT": np.ascontiguousarray(W_o[:, hs].T).astype(bf),
            "maskf": np.ascontiguousarray(mrow.reshape(KT, 128).T),
        })
    res = run_bass_kernel_spmd(_NC, in_maps, core_ids=list(range(NC_CORES)))
    out = np.empty((B, S, DM), np.float32)
    for b in range(B):
        out[b] = (res.results[2 * b]["outT"].astype(np.float32)
                  + res.results[2 * b + 1]["outT"].astype(np.float32)).T
    return out


# revision 8
# speedup vs baseline: 1.0577x; 1.0048x over previous
"""MultiHeadAttention Trainium2 kernel.

Sharding: 8 cores = 4 batches x 2 head-groups (8 heads each).
Each core computes, for its (batch b, head-group g):
  Q^T = Wq_g @ Xq^T, K^T = Wk_g @ Xk^T   (bf16 inputs/weights, f32 PSUM,
  [headdim, S] layout), V = Xv @ Wv_g^T  ([S, 512] layout, +ones col,
  mask-scaled, bf16), scores^T[k,q] per head (K=64 matmuls),
  e = exp(s/8) on ACT (PSUM->SBUF, bf16).
  attnV runs with e as the STATIONARY operand and V as the 65-wide moving
  operand (x~[q, hd] += e_chunk^T-weighted V), so the PE pays 65 cols
  instead of 512 per (head, k-tile, q-chunk): the softmax denominator is
  the ones column and lands per-PARTITION, so normalization is a plain
  per-partition reciprocal + scalar multiply on DVE.  The normalized
  x~[q, hd] staging tile is transposed back to [hd, q] layout with a
  cheap DMA xbar transpose (SP/HWDGE/DMA engines, zero PE cost), then
  out^T_partial = Wo_g^T.T @ x^T (bf16).
Host sums the two head-group partials per batch and transposes back.

Scheduling: the serial ramp is minimal (K m0 block0 + Q m0 block0 only,
~11us to the first exp); everything else (remaining K blocks/m-tiles,
per-head-pair V projection units, Q m-tiles, out-projections of the
previous q block) runs as deadline-scheduled side work inside the
phase-B supergroup loop, keeping ACT (the exp stream, the long pole)
fed as early and as continuously as possible.  attnV lags the exp
stream by 2 supergroups so V-projection side units have time to land.

Mask handling: V rows and the ones column are multiplied by mask (0/1), which
masks both the attnV numerator and the softmax denominator exactly.
"""
import contextlib
import os

import numpy as np
import ml_dtypes
import concourse.bass as bass  # noqa: F401
import concourse.tile as tile
from concourse import bacc, mybir
from concourse.bass_utils import run_bass_kernel_spmd

F32 = mybir.dt.float32
F32R = mybir.dt.float32r
BF16 = mybir.dt.bfloat16
EXP = mybir.ActivationFunctionType.Exp

B, S, DM = 4, 2048, 1024
H = 16
DK = 64
HLOC = 8              # heads per core
CW = HLOC * DK        # 512 local head dims per core
NC_CORES = 8
KT = S // 128         # 16 k-tiles
NB = S // 512         # 4 q/s blocks of 512
MT = CW // 128        # 4 m-tiles of local head dims
DT = DM // 128        # 8 contraction tiles over d_model
SCALE = 1.0 / np.sqrt(DK)
LAG = 2               # attnV supergroup lag behind the exp stream

_NC = None


def _env(k, d):
    return int(os.environ.get(k, d))


def _build():
    nc = bacc.Bacc()
    xqT = nc.dram_tensor("xqT", [DM, S], BF16, kind="ExternalInput")
    xkT = nc.dram_tensor("xkT", [DM, S], BF16, kind="ExternalInput")
    xvT = nc.dram_tensor("xvT", [DM, S], BF16, kind="ExternalInput")
    wqT = nc.dram_tensor("wqT", [DM, CW], BF16, kind="ExternalInput")
    wkT = nc.dram_tensor("wkT", [DM, CW], BF16, kind="ExternalInput")
    wvT = nc.dram_tensor("wvT", [DM, CW], BF16, kind="ExternalInput")
    woT = nc.dram_tensor("woT", [CW, DM], BF16, kind="ExternalInput")
    maskf = nc.dram_tensor("maskf", [128, KT], F32, kind="ExternalInput")
    outT = nc.dram_tensor("outT", [DM, S], BF16, kind="ExternalOutput")

    # DRAM views with the k-tile dim split out: row (k*128+p) -> [p, k, cols]
    xqv = xqT.rearrange("(k p) s -> p k s", p=128)
    xkv = xkT.rearrange("(k p) s -> p k s", p=128)
    xvv = xvT.rearrange("(k p) s -> p k s", p=128)
    wqv = wqT.rearrange("(k p) c -> p k c", p=128)
    wkv = wkT.rearrange("(k p) c -> p k c", p=128)
    wvv = wvT.rearrange("(k p) c -> p k c", p=128)
    wov = woT.rearrange("(k p) c -> p k c", p=128)
    outv = outT.rearrange("(m p) s -> p m s", p=128)

    with tile.TileContext(nc) as tc, contextlib.ExitStack() as ctx:
        persist = ctx.enter_context(tc.tile_pool(name="persist", bufs=1))

        # --- persistent tiles: mask, wo, Q^T/K^T slices, V ---
        m_sb = persist.tile([128, KT], F32)
        nc.sync.dma_start(m_sb[:], maskf[:])
        ones8 = persist.tile([128, HLOC], F32)
        nc.vector.memset(ones8[:], 1.0)
        warm = persist.tile([1, 1], F32)
        nc.scalar.activation(warm[:], ones8[0:1, 0:1], EXP, scale=1.0)
        q_tiles = {}   # (m, nb) -> [128, 512] bf16  (Q^T slice)
        k_tiles = {}
        for m in range(MT):
            for n in range(NB):
                q_tiles[(m, n)] = persist.tile(
                    [128, 512], BF16, tag=f"q{m}_{n}", name=f"q{m}_{n}")
                k_tiles[(m, n)] = persist.tile(
                    [128, 512], BF16, tag=f"k{m}_{n}", name=f"k{m}_{n}")
        v_sb = persist.tile([128, KT, HLOC, DK + 1], BF16, tag="v")
        wo_t = persist.tile([128, MT, DM], BF16, tag="wo")

        # weights persist through phase B (K/V/Q side units use them)
        wq_pool = ctx.enter_context(tc.tile_pool(name="wqp", bufs=1))
        xt = ctx.enter_context(tc.tile_pool(name="xt", bufs=_env("K_XT_BUFS", 10)))
        wq_sb = wq_pool.tile([128, DT, CW], BF16, tag="wq")
        wk_sb = wq_pool.tile([128, DT, CW], BF16, tag="wk")
        wv_sb = wq_pool.tile([128, DT, CW], BF16, tag="wv")
        dum = wq_pool.tile([128, 512], BF16, tag="dum")

        def dma_block(srcv, n, nm):
            """One batched DMA for an x block: [128, DT, 512] bf16 tile."""
            xts = xt.tile([128, DT, 512], BF16, tag="xt", name=f"{nm}{n}")
            nc.sync.dma_start(xts[:], srcv[:, :, n * 512:(n + 1) * 512])
            return xts

        # single projection m-group: 8 accumulating matmuls + DVE evac
        def proj_group(dst_tiles, w_sb, xts, n, m, pool, tag):
            ps = pool.tile([128, 512], F32, tag=tag, name=f"pj{n}_{m}_{tag}")
            for k in range(DT):
                nc.tensor.matmul(
                    ps[:], w_sb[:, k, m * 128:(m + 1) * 128],
                    xts[:, k, :], start=(k == 0), stop=(k == DT - 1))
            nc.vector.tensor_copy(dst_tiles[(m, n)][:], ps[:])

        def v_group_pair(n, sm, p, pool, tag):
            """V projection for k-tile t=n*4+sm, head pair p only (128 cols):
            V[kpos, 2 heads x 64] + mask scaling into v_sb."""
            t = n * 4 + sm
            ps = pool.tile([128, 128], F32, tag=tag, name=f"vp{t}_{p}")
            for k in range(DT):
                nc.tensor.matmul(
                    ps[:], xts_store[("v", n)][:, k, sm * 128:(sm + 1) * 128],
                    wv_sb[:, k, p * 128:(p + 1) * 128],
                    start=(k == 0), stop=(k == DT - 1))
            nc.vector.tensor_scalar_mul(
                v_sb[:, t, 2 * p:2 * p + 2, 0:DK],
                ps[:].rearrange("p (h d) -> p h d", h=2),
                m_sb[:, t:t + 1])
            nc.vector.tensor_scalar_mul(
                v_sb[:, t, 2 * p:2 * p + 2, DK:DK + 1], ones8[:, 0:2],
                m_sb[:, t:t + 1])

        # ---------------- Phase A: minimal serial ramp ----------------
        # DMA issue order = consumption order; the DMA engine pool is a
        # serial resource so order is everything.  PE warmup covers the
        # first DMAs and starts the pstate ramp.
        ctxA = contextlib.ExitStack()
        with ctxA:
            psA = ctxA.enter_context(tc.tile_pool(name="psA", bufs=4, space="PSUM"))
            nc.sync.dma_start(wk_sb[:, :, 0:128], wkv[:, :, 0:128])
            xk0 = dma_block(xkv, 0, "xk")
            nc.sync.dma_start(wq_sb[:, :, 0:128], wqv[:, :, 0:128])
            xq0 = dma_block(xqv, 0, "xq")
            xk_blocks = [xk0, dma_block(xkv, 1, "xk")]
            nc.sync.dma_start(wv_sb[:], wvv[:])
            xv_blocks = [dma_block(xvv, 0, "xv")]
            xv_blocks.append(dma_block(xvv, 1, "xv"))
            xk_blocks.append(dma_block(xkv, 2, "xk"))
            xv_blocks.append(dma_block(xvv, 2, "xv"))
            xk_blocks.append(dma_block(xkv, 3, "xk"))
            xv_blocks.append(dma_block(xvv, 3, "xv"))
            nc.sync.dma_start(wk_sb[:, :, 128:CW], wkv[:, :, 128:CW])
            nc.sync.dma_start(wq_sb[:, :, 128:CW], wqv[:, :, 128:CW])
            nc.sync.dma_start(wo_t[:], wov[:])

            nc.gpsimd.memset(dum[:], 0.0)
            for i in range(_env("K_WARM_MM", 2)):
                pw = psA.tile([128, 512], F32, tag="pa", name=f"warmmm{i}")
                for rep in range(_env("K_WARM_REP", 5)):
                    nc.tensor.matmul(pw[:], dum[:, 0:128], dum[:],
                                     start=(rep == 0), stop=True)
            proj_group(k_tiles, wk_sb, xk0, 0, 0, psA, "pa")
            proj_group(q_tiles, wq_sb, xq0, 0, 0, psA, "pa")

        # ---------------- Phase B: attention + out-proj ----------------
        QB = [(0, 512, 2), (512, 512, 2), (1024, 512, 2),
              (1536, 256, 4), (1792, 256, 4)]
        NQB = len(QB)
        with tc.tile_pool(name="ev", bufs=_env("K_EV_BUFS", 2 * (LAG + 1))) as ev, \
             tc.tile_pool(name="x", bufs=2) as xpool, \
             tc.tile_pool(name="xn", bufs=_env("K_XN_BUFS", 2)) as xnpool, \
             tc.tile_pool(name="small", bufs=_env("K_SMALL_BUFS", 4)) as small, \
             tc.tile_pool(name="o", bufs=2) as opool, \
             tc.tile_pool(name="psS", bufs=_env("K_PSS_BUFS", 3), space="PSUM") as psS, \
             tc.tile_pool(name="psX", bufs=_env("K_XO_BUFS", 2), space="PSUM") as psX:
            x_tiles = [xpool.tile([128, MT, 512], BF16, tag="xs",
                                  name=f"xs{i}") for i in range(2)]
            xn_stage = [xnpool.tile([128, 4, 512], BF16, tag="xn",
                                    name=f"xn{i}")
                        for i in range(_env("K_XN_BUFS", 2))]
            o_tiles = [opool.tile([128, DT, 512], BF16, tag="ob",
                                  name=f"ob{i}") for i in range(2)]

            def outproj_group(oqb, m, flush=False):
                col0, W, _ = QB[oqb]
                x_prev = x_tiles[oqb % 2]
                o_sb = o_tiles[oqb % 2]
                po = psS.tile([128, W], F32, tag="s", name=f"po{oqb}_{m}")
                for kk in range(MT):
                    nc.tensor.matmul(
                        po[:], wo_t[:, kk, m * 128:(m + 1) * 128],
                        x_prev[:, kk, 0:W], start=(kk == 0), stop=(kk == MT - 1))
                nc.vector.tensor_copy(o_sb[:, m, 0:W], po[:])
                if flush:
                    nc.sync.dma_start(
                        outv[:, :, col0:col0 + W], o_sb[:, :, 0:W])

            xts_store = {("v", n): xv_blocks[n] for n in range(NB)}
            xts_store[("q", 0)] = xq0

            def mk_vp(nn, sm, p):
                return ("mm", lambda: v_group_pair(nn, sm, p, psS, "s"))

            def mk_kg(m, b):
                return ("mm", lambda: proj_group(k_tiles, wk_sb, xk_blocks[b],
                                                 b, m, psS, "s"))

            def mk_qdma(nn):
                def f():
                    xts_store[("q", nn)] = dma_block(xqv, nn, "xq")
                return ("dma", f)

            def mk_qg(nn, m):
                return ("mm", lambda: proj_group(q_tiles, wq_sb,
                                                 xts_store[("q", nn)],
                                                 nn, m, psS, "s"))

            def mk_og(oqb, m, flush=False):
                return ("mm", lambda: outproj_group(oqb, m, flush))

            # (qb, p) -> [(min_sg, (kind, fn)), ...]
            # Block 0 hosts all remaining K m-tiles, per-pair V units and Q0
            # m-tiles, deadline-ordered: scores(p, sg) needs K m_p b(sg//2);
            # attnV at sg eats V t=sgw*(sg-LAG); pair p+1 needs K m_{p+1} b0
            # and Q0 m_{p+1} before it starts.
            side_work = {}

            def vp_sched(p, host_pair):
                """V units for pair p spread over hosting pair's sgs."""
                out = []
                for t in range(KT):
                    if host_pair == p:      # own pair: stay LAG sgs ahead
                        ms = max(1, t // 2)
                    else:                   # previous pair hosts: spread
                        ms = min(7, t // 2)
                    out.append((ms, mk_vp(t // 4, t % 4, p)))
                return out

            side_work[(0, 0)] = ([(1, mk_kg(0, 1)), (3, mk_kg(0, 2)),
                                  (5, mk_kg(0, 3)), (7, mk_kg(1, 0)),
                                  (7, mk_qg(0, 1))]
                                 + vp_sched(0, 0))
            side_work[(0, 1)] = ([(1, mk_kg(1, 1)), (3, mk_kg(1, 2)),
                                  (5, mk_kg(1, 3)), (7, mk_kg(2, 0)),
                                  (7, mk_qg(0, 2))]
                                 + vp_sched(1, 1))
            side_work[(0, 2)] = ([(2, mk_qdma(1)), (1, mk_kg(2, 1)),
                                  (3, mk_kg(2, 2)), (5, mk_kg(2, 3)),
                                  (7, mk_kg(3, 0)), (7, mk_qg(0, 3))]
                                 + vp_sched(2, 2))
            side_work[(0, 3)] = ([(1, mk_kg(3, 1)), (3, mk_kg(3, 2)),
                                  (5, mk_kg(3, 3)), (7, mk_qg(1, 0))]
                                 + vp_sched(3, 3))
            SIDE = {
                (1, 0): [(0, 'qdma', 2), (1, 'qg', 1, 1), (3, 'og', 0, 0),
                         (7, 'og', 0, 1), (4, 'qg', 2, 0)],
                (1, 1): [(0, 'qg', 1, 2), (3, 'og', 0, 2), (7, 'og', 0, 3),
                         (4, 'qg', 2, 1)],
                (1, 2): [(0, 'qg', 1, 3), (3, 'og', 0, 4), (7, 'og', 0, 5),
                         (4, 'qg', 2, 2)],
                (1, 3): [(3, 'og', 0, 6), (7, 'og', 0, 7), (4, 'qg', 2, 3)],
                (2, 0): [(0, 'qdma', 3), (3, 'og', 1, 0), (7, 'og', 1, 1),
                         (4, 'qg', 3, 0)],
                (2, 1): [(3, 'og', 1, 2), (7, 'og', 1, 3), (4, 'qg', 3, 1)],
                (2, 2): [(3, 'og', 1, 4), (7, 'og', 1, 5), (4, 'qg', 3, 2)],
                (2, 3): [(3, 'og', 1, 6), (7, 'og', 1, 7), (4, 'qg', 3, 3)],
                (3, 0): [(2, 'og', 2, 0)],
                (3, 1): [(1, 'og', 2, 2), (2, 'og', 2, 3)],
                (3, 2): [(1, 'og', 2, 4), (3, 'og', 2, 1)],
                (3, 3): [(1, 'og', 2, 5)],
                (4, 0): [(0, 'og', 2, 6), (2, 'og', 2, 7)],
                (4, 1): [(0, 'og', 3, 0), (1, 'og', 3, 1), (3, 'og', 3, 2)],
                (4, 2): [(0, 'og', 3, 3), (1, 'og', 3, 4), (3, 'og', 3, 5)],
                (4, 3): [(0, 'og', 3, 6), (1, 'og', 3, 7)],
            }
            for key, items in SIDE.items():
                lst = side_work.setdefault(key, [])
                for it in items:
                    if it[1] == 'qdma':
                        lst.append((it[0], mk_qdma(it[2])))
                    elif it[1] == 'qg':
                        lst.append((it[0], mk_qg(it[2], it[3])))
                    else:
                        lst.append((it[0], mk_og(it[2], it[3],
                                                 flush=(it[3] == DT - 1))))
            for key in side_work:
                side_work[key].sort(key=lambda it: it[0])

            MAXMM = _env("K_MAXMM", 1)
            MAXMM0 = _env("K_MAXMM0", 3)

            def side_step(qb, p, sg):
                work = side_work.get((qb, p))
                if not work:
                    return
                lim = MAXMM0 if qb == 0 else MAXMM
                did_mm = 0
                while work:
                    min_sg, (kind, fn) = work[0]
                    if min_sg > sg or (kind == "mm" and did_mm >= lim):
                        break
                    work.pop(0)
                    fn()
                    if kind == "mm":
                        did_mm += 1

            def side_flush(qb, p):
                for _, (kind, fn) in side_work.pop((qb, p), []):
                    fn()

            def attn_v(ps_x, h, sg, sgw, e_h, nqc):
                """Flipped attnV for supergroup sg: e chunks stationary,
                V [128, 65] moving, accumulating x~[q, hd|den] per qc."""
                for tt in range(sgw):
                    t = sg * sgw + tt
                    for qc in range(nqc):
                        # start=True zeroes the WHOLE psum bank, so only the
                        # very first matmul into this tile may set it; the
                        # other qc groups accumulate onto the zeroed bank.
                        nc.tensor.matmul(
                            ps_x[h][:, qc, :],
                            e_h[:, tt, qc * 128:(qc + 1) * 128],
                            v_sb[:, t, h, :],
                            start=(t == 0 and qc == 0), stop=(t == KT - 1))

            for qb in range(NQB):
                col0, W, sgw = QB[qb]
                nb = col0 // 512
                q0 = col0 % 512
                nsg = KT // sgw
                nqc = W // 128
                xn_sb = xn_stage[qb % len(xn_stage)]
                for p in range(MT):        # head pairs; pair p = heads 2p,2p+1
                    heads = (2 * p, 2 * p + 1)
                    ps_x = {h: psX.tile([128, nqc, DK + 1], F32, tag="xo",
                                        name=f"psx{qb}_{h}") for h in heads}
                    e_hist = {}
                    for sg in range(nsg):
                        side_step(qb, p, sg)
                        for h in heads:
                            hp = h % 2
                            ps_h = psS.tile([128, sgw, W], F32, tag="s",
                                            name=f"pss{qb}_{sg}_{h}")
                            for tt in range(sgw):
                                t = sg * sgw + tt
                                nc.tensor.matmul(
                                    ps_h[:, tt, :],
                                    k_tiles[(p, t // 4)][
                                        hp * 64:(hp + 1) * 64,
                                        (t % 4) * 128:(t % 4 + 1) * 128],
                                    q_tiles[(p, nb)][hp * 64:(hp + 1) * 64,
                                                     q0:q0 + W],
                                    start=True, stop=True)
                            e_sb = ev.tile([128, sgw, W], BF16, tag="e",
                                           name=f"e{qb}_{sg}_{h}")
                            nc.scalar.activation(e_sb[:], ps_h[:], EXP,
                                                 scale=float(SCALE))
                            e_hist[(sg, h)] = e_sb
                            if sg >= LAG:
                                attn_v(ps_x, h, sg - LAG, sgw,
                                       e_hist.pop((sg - LAG, h)), nqc)
                    side_flush(qb, p)
                    # drain last LAG supergroups + normalize into xn staging
                    for h in heads:
                        for j in range(LAG, 0, -1):
                            attn_v(ps_x, h, nsg - j, sgw,
                                   e_hist.pop((nsg - j, h)), nqc)
                        hp = h % 2
                        c0 = p * 128 + hp * 64
                        for qc in range(nqc):
                            r = small.tile([128, 1], F32, tag="r",
                                           name=f"r{qb}_{h}_{qc}")
                            nc.vector.reciprocal(r[:], ps_x[h][:, qc, DK:DK + 1])
                            nc.vector.tensor_scalar_mul(
                                xn_sb[:, qc, c0:c0 + 64],
                                ps_x[h][:, qc, 0:DK], r[:])
                    # x~ staged as [q, hd]; flip this pair's slice back to
                    # [hd, q] via the DMA xbar transpose (SP queue + DMA
                    # engines, no PE cost). Per-pair so the next block's
                    # outproj kk-matmuls find their deps already satisfied.
                    for qc in range(nqc):
                        nc.sync.dma_start_transpose(
                            x_tiles[qb % 2][:, p, qc * 128:(qc + 1) * 128],
                            xn_sb[:, qc, p * 128:(p + 1) * 128])
            # final out-projection for the last q block (its og side-work
            # can't ride a following block).
            oqb = NQB - 1
            col0, W, _ = QB[oqb]
            o_sb = o_tiles[oqb % 2]
            x_prev = x_tiles[oqb % 2]
            for m in range(DT):
                ms = slice(m * 128, (m + 1) * 128)
                po = psS.tile([128, W], F32, tag="s", name=f"pof{m}")
                for kk in range(MT):
                    nc.tensor.matmul(
                        po[:], wo_t[:, kk, ms], x_prev[:, kk, 0:W],
                        start=(kk == 0), stop=(kk == MT - 1))
                if m % 2:
                    nc.scalar.copy(o_sb[:, m, 0:W], po[:])
                else:
                    nc.vector.tensor_copy(o_sb[:, m, 0:W], po[:])
                if m == 3:
                    nc.sync.dma_start(
                        outv[:, 0:4, col0:col0 + W], o_sb[:, 0:4, 0:W])
                elif m == 6:
                    nc.sync.dma_start(
                        outv[:, 4:7, col0:col0 + W], o_sb[:, 4:7, 0:W])
            nc.sync.dma_start(
                outv[:, 7:8, col0:col0 + W], o_sb[:, 7:8, 0:W])
    nc.finalize()
    return nc


def kernel(query, key, value, mask, W_q, W_k, W_v, W_o):
    global _NC
    if _NC is None:
        _NC = _build()
    bf = ml_dtypes.bfloat16
    query = np.asarray(query, dtype=np.float32)
    key = np.asarray(key, dtype=np.float32)
    value = np.asarray(value, dtype=np.float32)
    W_q = np.asarray(W_q, dtype=np.float32)
    W_k = np.asarray(W_k, dtype=np.float32)
    W_v = np.asarray(W_v, dtype=np.float32)
    W_o = np.asarray(W_o, dtype=np.float32)
    mask = np.asarray(mask)

    in_maps = []
    for c in range(NC_CORES):
        b, g = divmod(c, 2)
        hs = slice(g * CW, (g + 1) * CW)
        mrow = (mask[b, 0, 0, :] != 0).astype(np.float32)
        in_maps.append({
            "xqT": np.ascontiguousarray(query[b].T).astype(bf),
            "xkT": np.ascontiguousarray(key[b].T).astype(bf),
            "xvT": np.ascontiguousarray(value[b].T).astype(bf),
            "wqT": np.ascontiguousarray(W_q[hs, :].T).astype(bf),
            "wkT": np.ascontiguousarray(W_k[hs, :].T).astype(bf),
            "wvT": np.ascontiguousarray(W_v[hs, :].T).astype(bf),
            "woT": np.ascontiguousarray(W_o[:, hs].T).astype(bf),
            "maskf": np.ascontiguousarray(mrow.reshape(KT, 128).T),
        })
    res = run_bass_kernel_spmd(_NC, in_maps, core_ids=list(range(NC_CORES)))
    out = np.empty((B, S, DM), np.float32)
    for b in range(B):
        out[b] = (res.results[2 * b]["outT"].astype(np.float32)
                  + res.results[2 * b + 1]["outT"].astype(np.float32)).T
    return out


# revision 11
# speedup vs baseline: 1.0654x; 1.0073x over previous
"""MultiHeadAttention Trainium2 kernel.

Sharding: 8 cores = 4 batches x 2 head-groups (8 heads each).
Each core computes, for its (batch b, head-group g):
  Q^T = Wq_g @ Xq^T, K^T = Wk_g @ Xk^T   (bf16 inputs/weights, f32 PSUM,
  [headdim, S] layout), V = Xv @ Wv_g^T  ([S, 512] layout, +ones col,
  mask-scaled, bf16), scores^T[k,q] per head (K=64 matmuls),
  e = exp(s/8) on ACT (PSUM->SBUF, bf16).
  attnV runs with e as the STATIONARY operand and V as the 65-wide moving
  operand (x~[q, hd] += e_chunk^T-weighted V), so the PE pays 65 cols
  instead of 512 per (head, k-tile, q-chunk): the softmax denominator is
  the ones column and lands per-PARTITION, so normalization is a plain
  per-partition reciprocal + scalar multiply on DVE.  The normalized
  x~[q, hd] staging tile is transposed back to [hd, q] layout with a
  cheap DMA xbar transpose (SP/HWDGE/DMA engines, zero PE cost), then
  out^T_partial = Wo_g^T.T @ x^T (bf16).
Host sums the two head-group partials per batch and transposes back.

Scheduling: the serial ramp is minimal (K m0 block0 + Q m0 block0 only,
~11us to the first exp); everything else (remaining K blocks/m-tiles,
per-head-pair V projection units, Q m-tiles, out-projections of the
previous q block) runs as deadline-scheduled side work inside the
phase-B supergroup loop, keeping ACT (the exp stream, the long pole)
fed as early and as continuously as possible.  attnV lags the exp
stream by 2 supergroups so V-projection side units have time to land.

Mask handling: V rows and the ones column are multiplied by mask (0/1), which
masks both the attnV numerator and the softmax denominator exactly.
"""
import contextlib
import os

import numpy as np
import ml_dtypes
import concourse.bass as bass  # noqa: F401
import concourse.tile as tile
from concourse import bacc, mybir
from concourse.bass_utils import run_bass_kernel_spmd

F32 = mybir.dt.float32
F32R = mybir.dt.float32r
BF16 = mybir.dt.bfloat16
EXP = mybir.ActivationFunctionType.Exp

B, S, DM = 4, 2048, 1024
H = 16
DK = 64
HLOC = 8              # heads per core
CW = HLOC * DK        # 512 local head dims per core
NC_CORES = 8
KT = S // 128         # 16 k-tiles
NB = S // 512         # 4 q/s blocks of 512
MT = CW // 128        # 4 m-tiles of local head dims
DT = DM // 128        # 8 contraction tiles over d_model
SCALE = 1.0 / np.sqrt(DK)

_NC = None


def _env(k, d):
    return int(os.environ.get(k, d))


LAG = _env("K_LAG", 2)   # attnV supergroup lag behind the exp stream


def _build():
    nc = bacc.Bacc()
    xqT = nc.dram_tensor("xqT", [DM, S], BF16, kind="ExternalInput")
    xkT = nc.dram_tensor("xkT", [DM, S], BF16, kind="ExternalInput")
    xvT = nc.dram_tensor("xvT", [DM, S], BF16, kind="ExternalInput")
    wqT = nc.dram_tensor("wqT", [DM, CW], BF16, kind="ExternalInput")
    wkT = nc.dram_tensor("wkT", [DM, CW], BF16, kind="ExternalInput")
    wvT = nc.dram_tensor("wvT", [DM, CW], BF16, kind="ExternalInput")
    woT = nc.dram_tensor("woT", [CW, DM], BF16, kind="ExternalInput")
    maskf = nc.dram_tensor("maskf", [128, KT], F32, kind="ExternalInput")
    outT = nc.dram_tensor("outT", [DM, S], BF16, kind="ExternalOutput")

    # DRAM views with the k-tile dim split out: row (k*128+p) -> [p, k, cols]
    xqv = xqT.rearrange("(k p) s -> p k s", p=128)
    xkv = xkT.rearrange("(k p) s -> p k s", p=128)
    xvv = xvT.rearrange("(k p) s -> p k s", p=128)
    wqv = wqT.rearrange("(k p) c -> p k c", p=128)
    wkv = wkT.rearrange("(k p) c -> p k c", p=128)
    wvv = wvT.rearrange("(k p) c -> p k c", p=128)
    wov = woT.rearrange("(k p) c -> p k c", p=128)
    outv = outT.rearrange("(m p) s -> p m s", p=128)

    with tile.TileContext(nc) as tc, contextlib.ExitStack() as ctx:
        persist = ctx.enter_context(tc.tile_pool(name="persist", bufs=1))

        # --- persistent tiles: mask, wo, Q^T/K^T slices, V ---
        m_sb = persist.tile([128, KT], F32)
        nc.sync.dma_start(m_sb[:], maskf[:])
        ones8 = persist.tile([128, HLOC], F32)
        nc.vector.memset(ones8[:], 1.0)
        warm = persist.tile([1, 1], F32)
        nc.scalar.activation(warm[:], ones8[0:1, 0:1], EXP, scale=1.0)
        q_tiles = {}   # (m, nb) -> [128, 512] bf16  (Q^T slice)
        k_tiles = {}
        for m in range(MT):
            for n in range(NB):
                q_tiles[(m, n)] = persist.tile(
                    [128, 512], BF16, tag=f"q{m}_{n}", name=f"q{m}_{n}")
                k_tiles[(m, n)] = persist.tile(
                    [128, 512], BF16, tag=f"k{m}_{n}", name=f"k{m}_{n}")
        v_sb = persist.tile([128, KT, HLOC, DK + 1], BF16, tag="v")
        wo_t = persist.tile([128, MT, DM], BF16, tag="wo")

        # weights persist through phase B (K/V/Q side units use them)
        wq_pool = ctx.enter_context(tc.tile_pool(name="wqp", bufs=1))
        xt = ctx.enter_context(tc.tile_pool(name="xt", bufs=_env("K_XT_BUFS", 10)))
        wq_sb = wq_pool.tile([128, DT, CW], BF16, tag="wq")
        wk_sb = wq_pool.tile([128, DT, CW], BF16, tag="wk")
        wv_sb = wq_pool.tile([128, DT, CW], BF16, tag="wv")
        dum = wq_pool.tile([128, 512], BF16, tag="dum")

        def dma_block(srcv, n, nm):
            """One batched DMA for an x block: [128, DT, 512] bf16 tile."""
            xts = xt.tile([128, DT, 512], BF16, tag="xt", name=f"{nm}{n}")
            nc.sync.dma_start(xts[:], srcv[:, :, n * 512:(n + 1) * 512])
            return xts

        # single projection m-group: 8 accumulating matmuls + DVE evac
        def proj_group(dst_tiles, w_sb, xts, n, m, pool, tag):
            ps = pool.tile([128, 512], F32, tag=tag, name=f"pj{n}_{m}_{tag}")
            for k in range(DT):
                nc.tensor.matmul(
                    ps[:], w_sb[:, k, m * 128:(m + 1) * 128],
                    xts[:, k, :], start=(k == 0), stop=(k == DT - 1))
            nc.vector.tensor_copy(dst_tiles[(m, n)][:], ps[:])

        def v_group_pair(n, sm, p, pool, tag):
            """V projection for k-tile t=n*4+sm, head pair p only (128 cols):
            V[kpos, 2 heads x 64] + mask scaling into v_sb."""
            t = n * 4 + sm
            ps = pool.tile([128, 128], F32, tag=tag, name=f"vp{t}_{p}")
            for k in range(DT):
                nc.tensor.matmul(
                    ps[:], xts_store[("v", n)][:, k, sm * 128:(sm + 1) * 128],
                    wv_sb[:, k, p * 128:(p + 1) * 128],
                    start=(k == 0), stop=(k == DT - 1))
            nc.vector.tensor_scalar_mul(
                v_sb[:, t, 2 * p:2 * p + 2, 0:DK],
                ps[:].rearrange("p (h d) -> p h d", h=2),
                m_sb[:, t:t + 1])
            nc.vector.tensor_scalar_mul(
                v_sb[:, t, 2 * p:2 * p + 2, DK:DK + 1], ones8[:, 0:2],
                m_sb[:, t:t + 1])

        # ---------------- Phase A: minimal serial ramp ----------------
        # DMA issue order = consumption order; the DMA engine pool is a
        # serial resource so order is everything.  PE warmup covers the
        # first DMAs and starts the pstate ramp.
        ctxA = contextlib.ExitStack()
        with ctxA:
            psA = ctxA.enter_context(tc.tile_pool(name="psA", bufs=4, space="PSUM"))
            # earliest-deadline-first DMA stream (the DMA pool is serial):
            # scores need wk/wq m0 + xk0/xq0; attnV (lag 2) streams xv;
            # K side units eat xk blocks; pair-1+ V units need wv cols 128+.
            nc.sync.dma_start(wk_sb[:, :, 0:128], wkv[:, :, 0:128])
            xk0 = dma_block(xkv, 0, "xk")
            nc.sync.dma_start(wq_sb[:, :, 0:128], wqv[:, :, 0:128])
            xq0 = dma_block(xqv, 0, "xq")
            xk_blocks = [xk0, dma_block(xkv, 1, "xk")]
            nc.sync.dma_start(wv_sb[:, :, 0:128], wvv[:, :, 0:128])
            xv_blocks = [dma_block(xvv, 0, "xv")]
            xk_blocks.append(dma_block(xkv, 2, "xk"))
            xv_blocks.append(dma_block(xvv, 1, "xv"))
            xk_blocks.append(dma_block(xkv, 3, "xk"))
            xv_blocks.append(dma_block(xvv, 2, "xv"))
            nc.sync.dma_start(wk_sb[:, :, 128:CW], wkv[:, :, 128:CW])
            nc.sync.dma_start(wq_sb[:, :, 128:CW], wqv[:, :, 128:CW])
            xv_blocks.append(dma_block(xvv, 3, "xv"))
            nc.sync.dma_start(wv_sb[:, :, 128:CW], wvv[:, :, 128:CW])
            nc.sync.dma_start(wo_t[:], wov[:])

            nc.gpsimd.memset(dum[:], 0.0)
            for i in range(_env("K_WARM_MM", 2)):
                pw = psA.tile([128, 512], F32, tag="pa", name=f"warmmm{i}")
                for rep in range(_env("K_WARM_REP", 5)):
                    nc.tensor.matmul(pw[:], dum[:, 0:128], dum[:],
                                     start=(rep == 0), stop=True)
            proj_group(k_tiles, wk_sb, xk0, 0, 0, psA, "pa")
            proj_group(q_tiles, wq_sb, xq0, 0, 0, psA, "pa")

        # ---------------- Phase B: attention + out-proj ----------------
        QB = [(0, 512, 2), (512, 512, 2), (1024, 512, 2),
              (1536, 256, 4), (1792, 256, 4)]
        NQB = len(QB)
        with tc.tile_pool(name="ev", bufs=_env("K_EV_BUFS", 2 * (LAG + 1))) as ev, \
             tc.tile_pool(name="x", bufs=2) as xpool, \
             tc.tile_pool(name="xn", bufs=_env("K_XN_BUFS", 2)) as xnpool, \
             tc.tile_pool(name="small", bufs=_env("K_SMALL_BUFS", 4)) as small, \
             tc.tile_pool(name="o", bufs=2) as opool, \
             tc.tile_pool(name="psS", bufs=_env("K_PSS_BUFS", 3), space="PSUM") as psS, \
             tc.tile_pool(name="psX", bufs=_env("K_XO_BUFS", 2), space="PSUM") as psX:
            x_tiles = [xpool.tile([128, MT, 512], BF16, tag="xs",
                                  name=f"xs{i}") for i in range(2)]
            xn_stage = [xnpool.tile([128, 4, 512], BF16, tag="xn",
                                    name=f"xn{i}")
                        for i in range(_env("K_XN_BUFS", 2))]
            o_tiles = [opool.tile([128, DT, 512], BF16, tag="ob",
                                  name=f"ob{i}") for i in range(2)]

            def outproj_group(oqb, m, flush=False):
                col0, W, _ = QB[oqb]
                x_prev = x_tiles[oqb % 2]
                o_sb = o_tiles[oqb % 2]
                po = psS.tile([128, W], F32, tag="s", name=f"po{oqb}_{m}")
                for kk in range(MT):
                    nc.tensor.matmul(
                        po[:], wo_t[:, kk, m * 128:(m + 1) * 128],
                        x_prev[:, kk, 0:W], start=(kk == 0), stop=(kk == MT - 1))
                nc.vector.tensor_copy(o_sb[:, m, 0:W], po[:])
                if flush:
                    nc.sync.dma_start(
                        outv[:, :, col0:col0 + W], o_sb[:, :, 0:W])

            xts_store = {("v", n): xv_blocks[n] for n in range(NB)}
            xts_store[("q", 0)] = xq0

            def mk_vp(nn, sm, p):
                return ("mm", lambda: v_group_pair(nn, sm, p, psS, "s"))

            def mk_kg(m, b):
                return ("mm", lambda: proj_group(k_tiles, wk_sb, xk_blocks[b],
                                                 b, m, psS, "s"))

            def mk_qdma(nn):
                def f():
                    xts_store[("q", nn)] = dma_block(xqv, nn, "xq")
                return ("dma", f)

            def mk_qg(nn, m):
                return ("mm", lambda: proj_group(q_tiles, wq_sb,
                                                 xts_store[("q", nn)],
                                                 nn, m, psS, "s"))

            def mk_og(oqb, m, flush=False):
                return ("mm", lambda: outproj_group(oqb, m, flush))

            # (qb, p) -> [(min_sg, (kind, fn)), ...]
            # Block 0 hosts all remaining K m-tiles, per-pair V units and Q0
            # m-tiles, deadline-ordered: scores(p, sg) needs K m_p b(sg//2);
            # attnV at sg eats V t=sgw*(sg-LAG); pair p+1 needs K m_{p+1} b0
            # and Q0 m_{p+1} before it starts.
            side_work = {}

            def vp_sched(p, host_pair):
                """V units for pair p spread over hosting pair's sgs."""
                out = []
                for t in range(KT):
                    if host_pair == p:      # own pair: stay LAG sgs ahead
                        ms = max(1, t // 2)
                    else:                   # previous pair hosts: spread
                        ms = min(7, t // 2)
                    out.append((ms, mk_vp(t // 4, t % 4, p)))
                return out

            side_work[(0, 0)] = ([(1, mk_kg(0, 1)), (3, mk_kg(0, 2)),
                                  (5, mk_kg(0, 3)), (7, mk_kg(1, 0)),
                                  (7, mk_qg(0, 1))]
                                 + vp_sched(0, 0))
            side_work[(0, 1)] = ([(1, mk_kg(1, 1)), (3, mk_kg(1, 2)),
                                  (5, mk_kg(1, 3)), (7, mk_kg(2, 0)),
                                  (7, mk_qg(0, 2))]
                                 + vp_sched(1, 1))
            side_work[(0, 2)] = ([(2, mk_qdma(1)), (1, mk_kg(2, 1)),
                                  (3, mk_kg(2, 2)), (5, mk_kg(2, 3)),
                                  (7, mk_kg(3, 0)), (7, mk_qg(0, 3))]
                                 + vp_sched(2, 2))
            side_work[(0, 3)] = ([(1, mk_kg(3, 1)), (3, mk_kg(3, 2)),
                                  (5, mk_kg(3, 3)), (7, mk_qg(1, 0))]
                                 + vp_sched(3, 3))
            SIDE = {
                (1, 0): [(0, 'qdma', 2), (1, 'qg', 1, 1), (3, 'og', 0, 0),
                         (7, 'og', 0, 1), (4, 'qg', 2, 0)],
                (1, 1): [(0, 'qg', 1, 2), (3, 'og', 0, 2), (7, 'og', 0, 3),
                         (4, 'qg', 2, 1)],
                (1, 2): [(0, 'qg', 1, 3), (3, 'og', 0, 4), (7, 'og', 0, 5),
                         (4, 'qg', 2, 2)],
                (1, 3): [(3, 'og', 0, 6), (7, 'og', 0, 7), (4, 'qg', 2, 3)],
                (2, 0): [(0, 'qdma', 3), (3, 'og', 1, 0), (7, 'og', 1, 1),
                         (4, 'qg', 3, 0)],
                (2, 1): [(3, 'og', 1, 2), (7, 'og', 1, 3), (4, 'qg', 3, 1)],
                (2, 2): [(3, 'og', 1, 4), (7, 'og', 1, 5), (4, 'qg', 3, 2)],
                (2, 3): [(3, 'og', 1, 6), (7, 'og', 1, 7), (4, 'qg', 3, 3)],
                (3, 0): [(2, 'og', 2, 0)],
                (3, 1): [(1, 'og', 2, 2), (2, 'og', 2, 3)],
                (3, 2): [(1, 'og', 2, 4), (3, 'og', 2, 1)],
                (3, 3): [(1, 'og', 2, 5)],
                (4, 0): [(0, 'og', 2, 6), (2, 'og', 2, 7)],
                (4, 1): [(0, 'og', 3, 0), (1, 'og', 3, 1), (3, 'og', 3, 2)],
                (4, 2): [(0, 'og', 3, 3), (1, 'og', 3, 4), (3, 'og', 3, 5)],
                (4, 3): [(0, 'og', 3, 6), (1, 'og', 3, 7)],
            }
            for key, items in SIDE.items():
                lst = side_work.setdefault(key, [])
                for it in items:
                    if it[1] == 'qdma':
                        lst.append((it[0], mk_qdma(it[2])))
                    elif it[1] == 'qg':
                        lst.append((it[0], mk_qg(it[2], it[3])))
                    else:
                        lst.append((it[0], mk_og(it[2], it[3],
                                                 flush=(it[3] == DT - 1))))
            for key in side_work:
                side_work[key].sort(key=lambda it: it[0])

            MAXMM = _env("K_MAXMM", 1)
            MAXMM0 = _env("K_MAXMM0", 4)

            def side_step(qb, p, sg):
                work = side_work.get((qb, p))
                if not work:
                    return
                lim = MAXMM0 if qb == 0 else MAXMM
                did_mm = 0
                while work:
                    min_sg, (kind, fn) = work[0]
                    if min_sg > sg or (kind == "mm" and did_mm >= lim):
                        break
                    work.pop(0)
                    fn()
                    if kind == "mm":
                        did_mm += 1

            def side_flush(qb, p):
                for _, (kind, fn) in side_work.pop((qb, p), []):
                    fn()

            def attn_v(ps_x, h, sg, sgw, e_h, nqc):
                """Flipped attnV for supergroup sg: e chunks stationary,
                V [128, 65] moving, accumulating x~[q, hd|den] per qc."""
                for tt in range(sgw):
                    t = sg * sgw + tt
                    for qc in range(nqc):
                        # start=True zeroes the WHOLE psum bank, so only the
                        # very first matmul into this tile may set it; the
                        # other qc groups accumulate onto the zeroed bank.
                        nc.tensor.matmul(
                            ps_x[h][:, qc, :],
                            e_h[:, tt, qc * 128:(qc + 1) * 128],
                            v_sb[:, t, h, :],
                            start=(t == 0 and qc == 0), stop=(t == KT - 1))

            for qb in range(NQB):
                col0, W, sgw = QB[qb]
                nb = col0 // 512
                q0 = col0 % 512
                nsg = KT // sgw
                nqc = W // 128
                xn_sb = xn_stage[qb % len(xn_stage)]
                for p in range(MT):        # head pairs; pair p = heads 2p,2p+1
                    heads = (2 * p, 2 * p + 1)
                    ps_x = {h: psX.tile([128, nqc, DK + 1], F32, tag="xo",
                                        name=f"psx{qb}_{h}") for h in heads}
                    e_hist = {}
                    for sg in range(nsg):
                        side_step(qb, p, sg)
                        for h in heads:
                            hp = h % 2
                            ps_h = psS.tile([128, sgw, W], F32, tag="s",
                                            name=f"pss{qb}_{sg}_{h}")
                            for tt in range(sgw):
                                t = sg * sgw + tt
                                nc.tensor.matmul(
                                    ps_h[:, tt, :],
                                    k_tiles[(p, t // 4)][
                                        hp * 64:(hp + 1) * 64,
                                        (t % 4) * 128:(t % 4 + 1) * 128],
                                    q_tiles[(p, nb)][hp * 64:(hp + 1) * 64,
                                                     q0:q0 + W],
                                    start=True, stop=True)
                            e_sb = ev.tile([128, sgw, W], BF16, tag="e",
                                           name=f"e{qb}_{sg}_{h}")
                            nc.scalar.activation(e_sb[:], ps_h[:], EXP,
                                                 scale=float(SCALE))
                            e_hist[(sg, h)] = e_sb
                            if sg >= LAG:
                                attn_v(ps_x, h, sg - LAG, sgw,
                                       e_hist.pop((sg - LAG, h)), nqc)
                    side_flush(qb, p)
                    # drain last LAG supergroups + normalize into xn staging
                    for h in heads:
                        for j in range(LAG, 0, -1):
                            attn_v(ps_x, h, nsg - j, sgw,
                                   e_hist.pop((nsg - j, h)), nqc)
                        hp = h % 2
                        c0 = p * 128 + hp * 64
                        for qc in range(nqc):
                            r = small.tile([128, 1], F32, tag="r",
                                           name=f"r{qb}_{h}_{qc}")
                            nc.vector.reciprocal(r[:], ps_x[h][:, qc, DK:DK + 1])
                            nc.vector.tensor_scalar_mul(
                                xn_sb[:, qc, c0:c0 + 64],
                                ps_x[h][:, qc, 0:DK], r[:])
                    # x~ staged as [q, hd]; flip this pair's slice back to
                    # [hd, q] via the DMA xbar transpose (SP queue + DMA
                    # engines, no PE cost). Per-pair so the next block's
                    # outproj kk-matmuls find their deps already satisfied.
                    for qc in range(nqc):
                        nc.sync.dma_start_transpose(
                            x_tiles[qb % 2][:, p, qc * 128:(qc + 1) * 128],
                            xn_sb[:, qc, p * 128:(p + 1) * 128])
            # final out-projection for the last q block (its og side-work
            # can't ride a following block).
            oqb = NQB - 1
            col0, W, _ = QB[oqb]
            o_sb = o_tiles[oqb % 2]
            x_prev = x_tiles[oqb % 2]
            for m in range(DT):
                ms = slice(m * 128, (m + 1) * 128)
                po = psS.tile([128, W], F32, tag="s", name=f"pof{m}")
                for kk in range(MT):
                    nc.tensor.matmul(
                        po[:], wo_t[:, kk, ms], x_prev[:, kk, 0:W],
                        start=(kk == 0), stop=(kk == MT - 1))
                if m % 2:
                    nc.scalar.copy(o_sb[:, m, 0:W], po[:])
                else:
                    nc.vector.tensor_copy(o_sb[:, m, 0:W], po[:])
                if m == 3:
                    nc.sync.dma_start(
                        outv[:, 0:4, col0:col0 + W], o_sb[:, 0:4, 0:W])
                elif m == 6:
                    nc.sync.dma_start(
                        outv[:, 4:7, col0:col0 + W], o_sb[:, 4:7, 0:W])
            nc.sync.dma_start(
                outv[:, 7:8, col0:col0 + W], o_sb[:, 7:8, 0:W])
    nc.finalize()
    return nc


def kernel(query, key, value, mask, W_q, W_k, W_v, W_o):
    global _NC
    if _NC is None:
        _NC = _build()
    bf = ml_dtypes.bfloat16
    query = np.asarray(query, dtype=np.float32)
    key = np.asarray(key, dtype=np.float32)
    value = np.asarray(value, dtype=np.float32)
    W_q = np.asarray(W_q, dtype=np.float32)
    W_k = np.asarray(W_k, dtype=np.float32)
    W_v = np.asarray(W_v, dtype=np.float32)
    W_o = np.asarray(W_o, dtype=np.float32)
    mask = np.asarray(mask)

    in_maps = []
    for c in range(NC_CORES):
        b, g = divmod(c, 2)
        hs = slice(g * CW, (g + 1) * CW)
        mrow = (mask[b, 0, 0, :] != 0).astype(np.float32)
        in_maps.append({
            "xqT": np.ascontiguousarray(query[b].T).astype(bf),
            "xkT": np.ascontiguousarray(key[b].T).astype(bf),
            "xvT": np.ascontiguousarray(value[b].T).astype(bf),
            "wqT": np.ascontiguousarray(W_q[hs, :].T).astype(bf),
            "wkT": np.ascontiguousarray(W_k[hs, :].T).astype(bf),
            "wvT": np.ascontiguousarray(W_v[hs, :].T).astype(bf),
            "woT": np.ascontiguousarray(W_o[:, hs].T).astype(bf),
            "maskf": np.ascontiguousarray(mrow.reshape(KT, 128).T),
        })
    res = run_bass_kernel_spmd(_NC, in_maps, core_ids=list(range(NC_CORES)))
    out = np.empty((B, S, DM), np.float32)
    for b in range(B):
        out[b] = (res.results[2 * b]["outT"].astype(np.float32)
                  + res.results[2 * b + 1]["outT"].astype(np.float32)).T
    return out


# revision 12
# speedup vs baseline: 1.0657x; 1.0002x over previous
"""MultiHeadAttention Trainium2 kernel.

Sharding: 8 cores = 4 batches x 2 head-groups (8 heads each).
Each core computes, for its (batch b, head-group g):
  Q^T = Wq_g @ Xq^T, K^T = Wk_g @ Xk^T   (bf16 inputs/weights, f32 PSUM,
  [headdim, S] layout), V = Xv @ Wv_g^T  ([S, 512] layout, +ones col,
  mask-scaled, bf16), scores^T[k,q] per head (K=64 matmuls),
  e = exp(s/8) on ACT (PSUM->SBUF, bf16).
  attnV runs with e as the STATIONARY operand and V as the 65-wide moving
  operand (x~[q, hd] += e_chunk^T-weighted V), so the PE pays 65 cols
  instead of 512 per (head, k-tile, q-chunk): the softmax denominator is
  the ones column and lands per-PARTITION, so normalization is a plain
  per-partition reciprocal + scalar multiply on DVE.  The normalized
  x~[q, hd] staging tile is transposed back to [hd, q] layout with a
  cheap DMA xbar transpose (SP/HWDGE/DMA engines, zero PE cost), then
  out^T_partial = Wo_g^T.T @ x^T (bf16).
Host sums the two head-group partials per batch and transposes back.

Scheduling: the serial ramp is minimal (K m0 block0 + Q m0 block0 only,
~11us to the first exp); everything else (remaining K blocks/m-tiles,
per-head-pair V projection units, Q m-tiles, out-projections of the
previous q block) runs as deadline-scheduled side work inside the
phase-B supergroup loop, keeping ACT (the exp stream, the long pole)
fed as early and as continuously as possible.  attnV lags the exp
stream by 2 supergroups so V-projection side units have time to land.

Mask handling: V rows and the ones column are multiplied by mask (0/1), which
masks both the attnV numerator and the softmax denominator exactly.
"""
import contextlib
import os

import numpy as np
import ml_dtypes
import concourse.bass as bass  # noqa: F401
import concourse.tile as tile
from concourse import bacc, mybir
from concourse.bass_utils import run_bass_kernel_spmd

F32 = mybir.dt.float32
F32R = mybir.dt.float32r
BF16 = mybir.dt.bfloat16
EXP = mybir.ActivationFunctionType.Exp

B, S, DM = 4, 2048, 1024
H = 16
DK = 64
HLOC = 8              # heads per core
CW = HLOC * DK        # 512 local head dims per core
NC_CORES = 8
KT = S // 128         # 16 k-tiles
NB = S // 512         # 4 q/s blocks of 512
MT = CW // 128        # 4 m-tiles of local head dims
DT = DM // 128        # 8 contraction tiles over d_model
SCALE = 1.0 / np.sqrt(DK)

_NC = None


def _env(k, d):
    return int(os.environ.get(k, d))


LAG = _env("K_LAG", 2)   # attnV supergroup lag behind the exp stream


def _build():
    nc = bacc.Bacc()
    xqT = nc.dram_tensor("xqT", [DM, S], BF16, kind="ExternalInput")
    xkT = nc.dram_tensor("xkT", [DM, S], BF16, kind="ExternalInput")
    xvT = nc.dram_tensor("xvT", [DM, S], BF16, kind="ExternalInput")
    wqT = nc.dram_tensor("wqT", [DM, CW], BF16, kind="ExternalInput")
    wkT = nc.dram_tensor("wkT", [DM, CW], BF16, kind="ExternalInput")
    wvT = nc.dram_tensor("wvT", [DM, CW], BF16, kind="ExternalInput")
    woT = nc.dram_tensor("woT", [CW, DM], BF16, kind="ExternalInput")
    maskf = nc.dram_tensor("maskf", [128, KT], F32, kind="ExternalInput")
    outT = nc.dram_tensor("outT", [DM, S], BF16, kind="ExternalOutput")

    # DRAM views with the k-tile dim split out: row (k*128+p) -> [p, k, cols]
    xqv = xqT.rearrange("(k p) s -> p k s", p=128)
    xkv = xkT.rearrange("(k p) s -> p k s", p=128)
    xvv = xvT.rearrange("(k p) s -> p k s", p=128)
    wqv = wqT.rearrange("(k p) c -> p k c", p=128)
    wkv = wkT.rearrange("(k p) c -> p k c", p=128)
    wvv = wvT.rearrange("(k p) c -> p k c", p=128)
    wov = woT.rearrange("(k p) c -> p k c", p=128)
    outv = outT.rearrange("(m p) s -> p m s", p=128)

    with tile.TileContext(nc) as tc, contextlib.ExitStack() as ctx:
        persist = ctx.enter_context(tc.tile_pool(name="persist", bufs=1))

        # --- persistent tiles: mask, wo, Q^T/K^T slices, V ---
        m_sb = persist.tile([128, KT], F32)
        nc.sync.dma_start(m_sb[:], maskf[:])
        ones8 = persist.tile([128, HLOC], F32)
        nc.vector.memset(ones8[:], 1.0)
        warm = persist.tile([1, 1], F32)
        nc.scalar.activation(warm[:], ones8[0:1, 0:1], EXP, scale=1.0)
        q_tiles = {}   # (m, nb) -> [128, 512] bf16  (Q^T slice)
        k_tiles = {}
        for m in range(MT):
            for n in range(NB):
                q_tiles[(m, n)] = persist.tile(
                    [128, 512], BF16, tag=f"q{m}_{n}", name=f"q{m}_{n}")
                k_tiles[(m, n)] = persist.tile(
                    [128, 512], BF16, tag=f"k{m}_{n}", name=f"k{m}_{n}")
        v_sb = persist.tile([128, KT, HLOC, DK + 1], BF16, tag="v")
        wo_t = persist.tile([128, MT, DM], BF16, tag="wo")

        # weights persist through phase B (K/V/Q side units use them)
        wq_pool = ctx.enter_context(tc.tile_pool(name="wqp", bufs=1))
        xt = ctx.enter_context(tc.tile_pool(name="xt", bufs=_env("K_XT_BUFS", 10)))
        wq_sb = wq_pool.tile([128, DT, CW], BF16, tag="wq")
        wk_sb = wq_pool.tile([128, DT, CW], BF16, tag="wk")
        wv_sb = wq_pool.tile([128, DT, CW], BF16, tag="wv")
        dum = wq_pool.tile([128, 512], BF16, tag="dum")

        def dma_block(srcv, n, nm):
            """One batched DMA for an x block: [128, DT, 512] bf16 tile."""
            xts = xt.tile([128, DT, 512], BF16, tag="xt", name=f"{nm}{n}")
            nc.sync.dma_start(xts[:], srcv[:, :, n * 512:(n + 1) * 512])
            return xts

        # single projection m-group: 8 accumulating matmuls + DVE evac
        def proj_group(dst_tiles, w_sb, xts, n, m, pool, tag):
            ps = pool.tile([128, 512], F32, tag=tag, name=f"pj{n}_{m}_{tag}")
            for k in range(DT):
                nc.tensor.matmul(
                    ps[:], w_sb[:, k, m * 128:(m + 1) * 128],
                    xts[:, k, :], start=(k == 0), stop=(k == DT - 1))
            nc.vector.tensor_copy(dst_tiles[(m, n)][:], ps[:])

        def v_group_pair(n, sm, p, pool, tag):
            """V projection for k-tile t=n*4+sm, head pair p only (128 cols):
            V[kpos, 2 heads x 64] + mask scaling into v_sb."""
            t = n * 4 + sm
            ps = pool.tile([128, 128], F32, tag=tag, name=f"vp{t}_{p}")
            for k in range(DT):
                nc.tensor.matmul(
                    ps[:], xts_store[("v", n)][:, k, sm * 128:(sm + 1) * 128],
                    wv_sb[:, k, p * 128:(p + 1) * 128],
                    start=(k == 0), stop=(k == DT - 1))
            nc.vector.tensor_scalar_mul(
                v_sb[:, t, 2 * p:2 * p + 2, 0:DK],
                ps[:].rearrange("p (h d) -> p h d", h=2),
                m_sb[:, t:t + 1])
            nc.vector.tensor_scalar_mul(
                v_sb[:, t, 2 * p:2 * p + 2, DK:DK + 1], ones8[:, 0:2],
                m_sb[:, t:t + 1])

        # ---------------- Phase A: minimal serial ramp ----------------
        # DMA issue order = consumption order; the DMA engine pool is a
        # serial resource so order is everything.  PE warmup covers the
        # first DMAs and starts the pstate ramp.
        ctxA = contextlib.ExitStack()
        with ctxA:
            psA = ctxA.enter_context(tc.tile_pool(name="psA", bufs=4, space="PSUM"))
            # earliest-deadline-first DMA stream (the DMA pool is serial):
            # scores need wk/wq m0 + xk0/xq0; attnV (lag 2) streams xv;
            # K side units eat xk blocks; pair-1+ V units need wv cols 128+.
            nc.sync.dma_start(wk_sb[:, :, 0:128], wkv[:, :, 0:128])
            xk0 = dma_block(xkv, 0, "xk")
            nc.sync.dma_start(wq_sb[:, :, 0:128], wqv[:, :, 0:128])
            xq0 = dma_block(xqv, 0, "xq")
            xk_blocks = [xk0, dma_block(xkv, 1, "xk")]
            nc.sync.dma_start(wv_sb[:, :, 0:128], wvv[:, :, 0:128])
            xv_blocks = [dma_block(xvv, 0, "xv")]
            xk_blocks.append(dma_block(xkv, 2, "xk"))
            xv_blocks.append(dma_block(xvv, 1, "xv"))
            xk_blocks.append(dma_block(xkv, 3, "xk"))
            xv_blocks.append(dma_block(xvv, 2, "xv"))
            nc.sync.dma_start(wk_sb[:, :, 128:CW], wkv[:, :, 128:CW])
            nc.sync.dma_start(wq_sb[:, :, 128:CW], wqv[:, :, 128:CW])
            xv_blocks.append(dma_block(xvv, 3, "xv"))
            nc.sync.dma_start(wv_sb[:, :, 128:CW], wvv[:, :, 128:CW])
            nc.sync.dma_start(wo_t[:], wov[:])

            nc.gpsimd.memset(dum[:], 0.0)
            for i in range(_env("K_WARM_MM", 2)):
                pw = psA.tile([128, 512], F32, tag="pa", name=f"warmmm{i}")
                for rep in range(_env("K_WARM_REP", 5)):
                    nc.tensor.matmul(pw[:], dum[:, 0:128], dum[:],
                                     start=(rep == 0), stop=True)
            proj_group(k_tiles, wk_sb, xk0, 0, 0, psA, "pa")
            proj_group(q_tiles, wq_sb, xq0, 0, 0, psA, "pa")

        # ---------------- Phase B: attention + out-proj ----------------
        QB = [(0, 512, 2), (512, 512, 2), (1024, 512, 2),
              (1536, 256, 4), (1792, 256, 4)]
        NQB = len(QB)
        with tc.tile_pool(name="ev", bufs=_env("K_EV_BUFS", 7)) as ev, \
             tc.tile_pool(name="x", bufs=2) as xpool, \
             tc.tile_pool(name="xn", bufs=_env("K_XN_BUFS", 2)) as xnpool, \
             tc.tile_pool(name="small", bufs=_env("K_SMALL_BUFS", 4)) as small, \
             tc.tile_pool(name="o", bufs=2) as opool, \
             tc.tile_pool(name="psS", bufs=_env("K_PSS_BUFS", 3), space="PSUM") as psS, \
             tc.tile_pool(name="psX", bufs=_env("K_XO_BUFS", 2), space="PSUM") as psX:
            x_tiles = [xpool.tile([128, MT, 512], BF16, tag="xs",
                                  name=f"xs{i}") for i in range(2)]
            xn_stage = [xnpool.tile([128, 4, 512], BF16, tag="xn",
                                    name=f"xn{i}")
                        for i in range(_env("K_XN_BUFS", 2))]
            o_tiles = [opool.tile([128, DT, 512], BF16, tag="ob",
                                  name=f"ob{i}") for i in range(2)]

            def outproj_group(oqb, m, flush=False):
                col0, W, _ = QB[oqb]
                x_prev = x_tiles[oqb % 2]
                o_sb = o_tiles[oqb % 2]
                po = psS.tile([128, W], F32, tag="s", name=f"po{oqb}_{m}")
                for kk in range(MT):
                    nc.tensor.matmul(
                        po[:], wo_t[:, kk, m * 128:(m + 1) * 128],
                        x_prev[:, kk, 0:W], start=(kk == 0), stop=(kk == MT - 1))
                nc.vector.tensor_copy(o_sb[:, m, 0:W], po[:])
                if flush:
                    nc.sync.dma_start(
                        outv[:, :, col0:col0 + W], o_sb[:, :, 0:W])

            xts_store = {("v", n): xv_blocks[n] for n in range(NB)}
            xts_store[("q", 0)] = xq0

            def mk_vp(nn, sm, p):
                return ("mm", lambda: v_group_pair(nn, sm, p, psS, "s"))

            def mk_kg(m, b):
                return ("mm", lambda: proj_group(k_tiles, wk_sb, xk_blocks[b],
                                                 b, m, psS, "s"))

            def mk_qdma(nn):
                def f():
                    xts_store[("q", nn)] = dma_block(xqv, nn, "xq")
                return ("dma", f)

            def mk_qg(nn, m):
                return ("mm", lambda: proj_group(q_tiles, wq_sb,
                                                 xts_store[("q", nn)],
                                                 nn, m, psS, "s"))

            def mk_og(oqb, m, flush=False):
                return ("mm", lambda: outproj_group(oqb, m, flush))

            # (qb, p) -> [(min_sg, (kind, fn)), ...]
            # Block 0 hosts all remaining K m-tiles, per-pair V units and Q0
            # m-tiles, deadline-ordered: scores(p, sg) needs K m_p b(sg//2);
            # attnV at sg eats V t=sgw*(sg-LAG); pair p+1 needs K m_{p+1} b0
            # and Q0 m_{p+1} before it starts.
            side_work = {}

            def vp_sched(p, host_pair):
                """V units for pair p spread over hosting pair's sgs."""
                out = []
                for t in range(KT):
                    if host_pair == p:      # own pair: stay LAG sgs ahead
                        ms = max(1, t // 2)
                    else:                   # previous pair hosts: spread
                        ms = min(7, t // 2)
                    out.append((ms, mk_vp(t // 4, t % 4, p)))
                return out

            side_work[(0, 0)] = ([(1, mk_kg(0, 1)), (3, mk_kg(0, 2)),
                                  (5, mk_kg(0, 3)), (7, mk_kg(1, 0)),
                                  (7, mk_qg(0, 1))]
                                 + vp_sched(0, 0))
            side_work[(0, 1)] = ([(1, mk_kg(1, 1)), (3, mk_kg(1, 2)),
                                  (5, mk_kg(1, 3)), (7, mk_kg(2, 0)),
                                  (7, mk_qg(0, 2))]
                                 + vp_sched(1, 1))
            side_work[(0, 2)] = ([(2, mk_qdma(1)), (1, mk_kg(2, 1)),
                                  (3, mk_kg(2, 2)), (5, mk_kg(2, 3)),
                                  (7, mk_kg(3, 0)), (7, mk_qg(0, 3))]
                                 + vp_sched(2, 2))
            side_work[(0, 3)] = ([(1, mk_kg(3, 1)), (3, mk_kg(3, 2)),
                                  (5, mk_kg(3, 3)), (7, mk_qg(1, 0))]
                                 + vp_sched(3, 3))
            SIDE = {
                (1, 0): [(0, 'qdma', 2), (1, 'qg', 1, 1), (3, 'og', 0, 0),
                         (7, 'og', 0, 1), (4, 'qg', 2, 0)],
                (1, 1): [(0, 'qg', 1, 2), (3, 'og', 0, 2), (7, 'og', 0, 3),
                         (4, 'qg', 2, 1)],
                (1, 2): [(0, 'qg', 1, 3), (3, 'og', 0, 4), (7, 'og', 0, 5),
                         (4, 'qg', 2, 2)],
                (1, 3): [(3, 'og', 0, 6), (7, 'og', 0, 7), (4, 'qg', 2, 3)],
                (2, 0): [(0, 'qdma', 3), (3, 'og', 1, 0), (7, 'og', 1, 1),
                         (4, 'qg', 3, 0)],
                (2, 1): [(3, 'og', 1, 2), (7, 'og', 1, 3), (4, 'qg', 3, 1)],
                (2, 2): [(3, 'og', 1, 4), (7, 'og', 1, 5), (4, 'qg', 3, 2)],
                (2, 3): [(3, 'og', 1, 6), (7, 'og', 1, 7), (4, 'qg', 3, 3)],
                (3, 0): [(2, 'og', 2, 0)],
                (3, 1): [(1, 'og', 2, 2), (2, 'og', 2, 3)],
                (3, 2): [(1, 'og', 2, 4), (3, 'og', 2, 1)],
                (3, 3): [(1, 'og', 2, 5)],
                (4, 0): [(0, 'og', 2, 6), (2, 'og', 2, 7)],
                (4, 1): [(0, 'og', 3, 0), (1, 'og', 3, 1), (3, 'og', 3, 2)],
                (4, 2): [(0, 'og', 3, 3), (1, 'og', 3, 4), (3, 'og', 3, 5)],
                (4, 3): [(0, 'og', 3, 6), (1, 'og', 3, 7)],
            }
            for key, items in SIDE.items():
                lst = side_work.setdefault(key, [])
                for it in items:
                    if it[1] == 'qdma':
                        lst.append((it[0], mk_qdma(it[2])))
                    elif it[1] == 'qg':
                        lst.append((it[0], mk_qg(it[2], it[3])))
                    else:
                        lst.append((it[0], mk_og(it[2], it[3],
                                                 flush=(it[3] == DT - 1))))
            for key in side_work:
                side_work[key].sort(key=lambda it: it[0])

            MAXMM = _env("K_MAXMM", 1)
            MAXMM0 = _env("K_MAXMM0", 5)

            def side_step(qb, p, sg):
                work = side_work.get((qb, p))
                if not work:
                    return
                lim = MAXMM0 if qb == 0 else MAXMM
                did_mm = 0
                while work:
                    min_sg, (kind, fn) = work[0]
                    if min_sg > sg or (kind == "mm" and did_mm >= lim):
                        break
                    work.pop(0)
                    fn()
                    if kind == "mm":
                        did_mm += 1

            def side_flush(qb, p):
                for _, (kind, fn) in side_work.pop((qb, p), []):
                    fn()

            def attn_v(ps_x, h, sg, sgw, e_h, nqc):
                """Flipped attnV for supergroup sg: e chunks stationary,
                V [128, 65] moving, accumulating x~[q, hd|den] per qc."""
                for tt in range(sgw):
                    t = sg * sgw + tt
                    for qc in range(nqc):
                        # start=True zeroes the WHOLE psum bank, so only the
                        # very first matmul into this tile may set it; the
                        # other qc groups accumulate onto the zeroed bank.
                        nc.tensor.matmul(
                            ps_x[h][:, qc, :],
                            e_h[:, tt, qc * 128:(qc + 1) * 128],
                            v_sb[:, t, h, :],
                            start=(t == 0 and qc == 0), stop=(t == KT - 1))

            for qb in range(NQB):
                col0, W, sgw = QB[qb]
                nb = col0 // 512
                q0 = col0 % 512
                nsg = KT // sgw
                nqc = W // 128
                xn_sb = xn_stage[qb % len(xn_stage)]
                for p in range(MT):        # head pairs; pair p = heads 2p,2p+1
                    heads = (2 * p, 2 * p + 1)
                    ps_x = {h: psX.tile([128, nqc, DK + 1], F32, tag="xo",
                                        name=f"psx{qb}_{h}") for h in heads}
                    e_hist = {}
                    for sg in range(nsg):
                        side_step(qb, p, sg)
                        for h in heads:
                            hp = h % 2
                            ps_h = psS.tile([128, sgw, W], F32, tag="s",
                                            name=f"pss{qb}_{sg}_{h}")
                            for tt in range(sgw):
                                t = sg * sgw + tt
                                nc.tensor.matmul(
                                    ps_h[:, tt, :],
                                    k_tiles[(p, t // 4)][
                                        hp * 64:(hp + 1) * 64,
                                        (t % 4) * 128:(t % 4 + 1) * 128],
                                    q_tiles[(p, nb)][hp * 64:(hp + 1) * 64,
                                                     q0:q0 + W],
                                    start=True, stop=True)
                            e_sb = ev.tile([128, sgw, W], BF16, tag="e",
                                           name=f"e{qb}_{sg}_{h}")
                            nc.scalar.activation(e_sb[:], ps_h[:], EXP,
                                                 scale=float(SCALE))
                            e_hist[(sg, h)] = e_sb
                            if sg >= LAG:
                                attn_v(ps_x, h, sg - LAG, sgw,
                                       e_hist.pop((sg - LAG, h)), nqc)
                    side_flush(qb, p)
                    # drain last LAG supergroups + normalize into xn staging
                    for h in heads:
                        for j in range(LAG, 0, -1):
                            attn_v(ps_x, h, nsg - j, sgw,
                                   e_hist.pop((nsg - j, h)), nqc)
                        hp = h % 2
                        c0 = p * 128 + hp * 64
                        for qc in range(nqc):
                            r = small.tile([128, 1], F32, tag="r",
                                           name=f"r{qb}_{h}_{qc}")
                            nc.vector.reciprocal(r[:], ps_x[h][:, qc, DK:DK + 1])
                            nc.vector.tensor_scalar_mul(
                                xn_sb[:, qc, c0:c0 + 64],
                                ps_x[h][:, qc, 0:DK], r[:])
                    # x~ staged as [q, hd]; flip this pair's slice back to
                    # [hd, q] via the DMA xbar transpose (SP queue + DMA
                    # engines, no PE cost). Per-pair so the next block's
                    # outproj kk-matmuls find their deps already satisfied.
                    for qc in range(nqc):
                        nc.sync.dma_start_transpose(
                            x_tiles[qb % 2][:, p, qc * 128:(qc + 1) * 128],
                            xn_sb[:, qc, p * 128:(p + 1) * 128])
            # final out-projection for the last q block (its og side-work
            # can't ride a following block).
            oqb = NQB - 1
            col0, W, _ = QB[oqb]
            o_sb = o_tiles[oqb % 2]
            x_prev = x_tiles[oqb % 2]
            for m in range(DT):
                ms = slice(m * 128, (m + 1) * 128)
                po = psS.tile([128, W], F32, tag="s", name=f"pof{m}")
                for kk in range(MT):
                    nc.tensor.matmul(
                        po[:], wo_t[:, kk, ms], x_prev[:, kk, 0:W],
                        start=(kk == 0), stop=(kk == MT - 1))
                if m % 2:
                    nc.scalar.copy(o_sb[:, m, 0:W], po[:])
                else:
                    nc.vector.tensor_copy(o_sb[:, m, 0:W], po[:])
                if m == 3:
                    nc.sync.dma_start(
                        outv[:, 0:4, col0:col0 + W], o_sb[:, 0:4, 0:W])
                elif m == 6:
                    nc.sync.dma_start(
                        outv[:, 4:7, col0:col0 + W], o_sb[:, 4:7, 0:W])
            nc.sync.dma_start(
                outv[:, 7:8, col0:col0 + W], o_sb[:, 7:8, 0:W])
    nc.finalize()
    return nc


def kernel(query, key, value, mask, W_q, W_k, W_v, W_o):
    global _NC
    if _NC is None:
        _NC = _build()
    bf = ml_dtypes.bfloat16
    query = np.asarray(query, dtype=np.float32)
    key = np.asarray(key, dtype=np.float32)
    value = np.asarray(value, dtype=np.float32)
    W_q = np.asarray(W_q, dtype=np.float32)
    W_k = np.asarray(W_k, dtype=np.float32)
    W_v = np.asarray(W_v, dtype=np.float32)
    W_o = np.asarray(W_o, dtype=np.float32)
    mask = np.asarray(mask)

    in_maps = []
    for c in range(NC_CORES):
        b, g = divmod(c, 2)
        hs = slice(g * CW, (g + 1) * CW)
        mrow = (mask[b, 0, 0, :] != 0).astype(np.float32)
        in_maps.append({
            "xqT": np.ascontiguousarray(query[b].T).astype(bf),
            "xkT": np.ascontiguousarray(key[b].T).astype(bf),
            "xvT": np.ascontiguousarray(value[b].T).astype(bf),
            "wqT": np.ascontiguousarray(W_q[hs, :].T).astype(bf),
            "wkT": np.ascontiguousarray(W_k[hs, :].T).astype(bf),
            "wvT": np.ascontiguousarray(W_v[hs, :].T).astype(bf),
            "woT": np.ascontiguousarray(W_o[:, hs].T).astype(bf),
            "maskf": np.ascontiguousarray(mrow.reshape(KT, 128).T),
        })
    res = run_bass_kernel_spmd(_NC, in_maps, core_ids=list(range(NC_CORES)))
    out = np.empty((B, S, DM), np.float32)
    for b in range(B):
        out[b] = (res.results[2 * b]["outT"].astype(np.float32)
                  + res.results[2 * b + 1]["outT"].astype(np.float32)).T
    return out


# revision 23
# speedup vs baseline: 1.1535x; 1.0825x over previous
"""MultiHeadAttention Trainium2 kernel.

Sharding: 8 cores = 4 batches x 2 head-groups (8 heads each).
Each core computes, for its (batch b, head-group g):
  Q^T = Wq_g @ Xq^T, K^T = Wk_g @ Xk^T   (bf16 inputs/weights, f32 PSUM,
  [headdim, S] layout), V = Xv @ Wv_g^T  ([S, 512] layout, +ones col,
  mask-scaled, bf16), scores^T[k,q] per head (K=64 matmuls),
  e = exp(s/8) on ACT (PSUM->SBUF, bf16).
  attnV runs with e as the STATIONARY operand and V as the 65-wide moving
  operand (x~[q, hd] += e_chunk^T-weighted V), so the PE pays 65 cols
  instead of 512 per (head, k-tile, q-chunk): the softmax denominator is
  the ones column and lands per-PARTITION, so normalization is a plain
  per-partition reciprocal + scalar multiply on DVE.  The normalized
  x~[q, hd] staging tile is transposed back to [hd, q] layout with a
  cheap DMA xbar transpose (SP/HWDGE/DMA engines, zero PE cost), then
  out^T_partial = Wo_g^T.T @ x^T (bf16).
Host sums the two head-group partials per batch and transposes back.

Scheduling: the serial ramp is minimal (K m0 block0 + Q m0 block0 only,
~11us to the first exp); everything else (remaining K blocks/m-tiles,
per-head-pair V projection units, Q m-tiles, out-projections of the
previous q block) runs as deadline-scheduled side work inside the
phase-B supergroup loop, keeping ACT (the exp stream, the long pole)
fed as early and as continuously as possible.  attnV lags the exp
stream by 2 supergroups so V-projection side units have time to land.

Mask handling: V rows and the ones column are multiplied by mask (0/1), which
masks both the attnV numerator and the softmax denominator exactly.
"""
import contextlib
import os

import numpy as np
import ml_dtypes
import concourse.bass as bass  # noqa: F401
import concourse.tile as tile
from concourse import bacc, mybir
from concourse.bass_utils import run_bass_kernel_spmd

F32 = mybir.dt.float32
F32R = mybir.dt.float32r
BF16 = mybir.dt.bfloat16
EXP = mybir.ActivationFunctionType.Exp

B, S, DM = 4, 2048, 1024
H = 16
DK = 64
HLOC = 8              # heads per core
CW = HLOC * DK        # 512 local head dims per core
NC_CORES = 8
KT = S // 128         # 16 k-tiles
NB = S // 512         # 4 q/s blocks of 512
MT = CW // 128        # 4 m-tiles of local head dims
DT = DM // 128        # 8 contraction tiles over d_model
SCALE = 1.0 / np.sqrt(DK)

_NC = None


def _env(k, d):
    return int(os.environ.get(k, d))


LAG = _env("K_LAG", 3)   # attnV supergroup lag behind the exp stream


def _build():
    nc = bacc.Bacc()
    xqT = nc.dram_tensor("xqT", [DM, S], BF16, kind="ExternalInput")
    xkT = nc.dram_tensor("xkT", [DM, S], BF16, kind="ExternalInput")
    xvT = nc.dram_tensor("xvT", [DM, S], BF16, kind="ExternalInput")
    # m-major weight layout [p, m, k, j]: each m-tile's [DT,128] block is
    # contiguous per partition row, so m-sliced DMAs run at full rate
    wqT = nc.dram_tensor("wqT", [128, MT, DT, 128], BF16, kind="ExternalInput")
    wkT = nc.dram_tensor("wkT", [128, MT, DT, 128], BF16, kind="ExternalInput")
    wvT = nc.dram_tensor("wvT", [128, MT, DT, 128], BF16, kind="ExternalInput")
    woT = nc.dram_tensor("woT", [CW, DM], BF16, kind="ExternalInput")
    maskf = nc.dram_tensor("maskf", [128, KT], F32, kind="ExternalInput")
    outT = nc.dram_tensor("outT", [DM, S], BF16, kind="ExternalOutput")

    # DRAM views with the k-tile dim split out: row (k*128+p) -> [p, k, cols]
    xqv = xqT.rearrange("(k p) s -> p k s", p=128)
    xkv = xkT.rearrange("(k p) s -> p k s", p=128)
    xvv = xvT.rearrange("(k p) s -> p k s", p=128)
    wov = woT.rearrange("(k p) c -> p k c", p=128)
    outv = outT.rearrange("(m p) s -> p m s", p=128)

    with tile.TileContext(nc) as tc, contextlib.ExitStack() as ctx:
        persist = ctx.enter_context(tc.tile_pool(name="persist", bufs=1))

        # --- persistent tiles: mask, wo, Q^T/K^T slices, V ---
        m_sb = persist.tile([128, KT], F32)
        nc.sync.dma_start(m_sb[:], maskf[:])
        ones8 = persist.tile([128, HLOC], F32)
        nc.vector.memset(ones8[:], 1.0)
        warm = persist.tile([1, 1], F32)
        nc.scalar.activation(warm[:], ones8[0:1, 0:1], EXP, scale=1.0)
        q_tiles = {}   # (m, nb) -> [128, 512] bf16  (Q^T slice)
        k_tiles = {}
        for m in range(MT):
            for n in range(NB):
                q_tiles[(m, n)] = persist.tile(
                    [128, 512], BF16, tag=f"q{m}_{n}", name=f"q{m}_{n}")
                k_tiles[(m, n)] = persist.tile(
                    [128, 512], BF16, tag=f"k{m}_{n}", name=f"k{m}_{n}")
        v_sb = persist.tile([128, KT, HLOC, DK + 1], BF16, tag="v")
        wo_t = persist.tile([128, MT, DM], BF16, tag="wo")

        # weights persist through phase B (K/V/Q side units use them)
        wq_pool = ctx.enter_context(tc.tile_pool(name="wqp", bufs=1))
        xt = ctx.enter_context(tc.tile_pool(name="xt", bufs=_env("K_XT_BUFS", 10)))
        wq_sb = wq_pool.tile([128, MT, DT, 128], BF16, tag="wq")
        wk_sb = wq_pool.tile([128, MT, DT, 128], BF16, tag="wk")
        wv_sb = wq_pool.tile([128, MT, DT, 128], BF16, tag="wv")
        dum = wq_pool.tile([128, 512], BF16, tag="dum")

        def dma_block(srcv, n, nm):
            """One batched DMA for an x block: [128, DT, 512] bf16 tile."""
            xts = xt.tile([128, DT, 512], BF16, tag="xt", name=f"{nm}{n}")
            nc.sync.dma_start(xts[:], srcv[:, :, n * 512:(n + 1) * 512])
            return xts

        # single projection m-group: 8 accumulating matmuls + DVE evac
        def proj_group(dst_tiles, w_sb, xts, n, m, pool, tag):
            ps = pool.tile([128, 512], F32, tag=tag, name=f"pj{n}_{m}_{tag}")
            for k in range(DT):
                nc.tensor.matmul(
                    ps[:], w_sb[:, m, k, :],
                    xts[:, k, :], start=(k == 0), stop=(k == DT - 1))
            nc.vector.tensor_copy(dst_tiles[(m, n)][:], ps[:])

        def v_group_pair(n, sm, p, pool, tag):
            """V projection for k-tile t=n*4+sm, head pair p only (128 cols):
            V[kpos, 2 heads x 64] + mask scaling into v_sb."""
            t = n * 4 + sm
            ps = pool.tile([128, 128], F32, tag=tag, name=f"vp{t}_{p}")
            for k in range(DT):
                nc.tensor.matmul(
                    ps[:], xts_store[("v", n)][:, k, sm * 128:(sm + 1) * 128],
                    wv_sb[:, p, k, :],
                    start=(k == 0), stop=(k == DT - 1))
            nc.vector.tensor_scalar_mul(
                v_sb[:, t, 2 * p:2 * p + 2, 0:DK],
                ps[:].rearrange("p (h d) -> p h d", h=2),
                m_sb[:, t:t + 1])
            nc.vector.tensor_scalar_mul(
                v_sb[:, t, 2 * p:2 * p + 2, DK:DK + 1], ones8[:, 0:2],
                m_sb[:, t:t + 1])

        # ---------------- Phase A: minimal serial ramp ----------------
        # DMA issue order = consumption order; the DMA engine pool is a
        # serial resource so order is everything.  PE warmup covers the
        # first DMAs and starts the pstate ramp.
        ctxA = contextlib.ExitStack()
        with ctxA:
            psA = ctxA.enter_context(tc.tile_pool(name="psA", bufs=4, space="PSUM"))
            # earliest-deadline-first DMA stream (the DMA pool is serial):
            # scores need wk/wq m0 + xk0/xq0; attnV (lag 2) streams xv;
            # K side units eat xk blocks; pair-1+ V units need wv cols 128+.
            hh = DT // 2
            nc.sync.dma_start(wk_sb[:, 0], wkT[:, 0])
            xk0 = xt.tile([128, DT, 512], BF16, tag="xt", name="xk0")
            nc.sync.dma_start(xk0[:, 0:hh, :], xkv[:, 0:hh, 0:512])
            nc.sync.dma_start(xk0[:, hh:DT, :], xkv[:, hh:DT, 0:512])
            nc.sync.dma_start(wq_sb[:, 0], wqT[:, 0])
            xq0 = xt.tile([128, DT, 512], BF16, tag="xt", name="xq0")
            nc.sync.dma_start(xq0[:, 0:hh, :], xqv[:, 0:hh, 0:512])
            nc.sync.dma_start(xq0[:, hh:DT, :], xqv[:, hh:DT, 0:512])
            xk_blocks = [xk0, dma_block(xkv, 1, "xk")]
            nc.sync.dma_start(wv_sb[:, 0], wvT[:, 0])
            xv_blocks = [dma_block(xvv, 0, "xv")]
            xk_blocks.append(dma_block(xkv, 2, "xk"))
            xv_blocks.append(dma_block(xvv, 1, "xv"))
            xk_blocks.append(dma_block(xkv, 3, "xk"))
            xv_blocks.append(dma_block(xvv, 2, "xv"))
            nc.sync.dma_start(wk_sb[:, 1:MT], wkT[:, 1:MT])
            nc.sync.dma_start(wq_sb[:, 1:MT], wqT[:, 1:MT])
            xv_blocks.append(dma_block(xvv, 3, "xv"))
            nc.sync.dma_start(wv_sb[:, 1:MT], wvT[:, 1:MT])
            nc.sync.dma_start(wo_t[:], wov[:])

            nc.gpsimd.memset(dum[:], 0.0)
            for i in range(_env("K_WARM_MM", 2)):
                pw = psA.tile([128, 512], F32, tag="pa", name=f"warmmm{i}")
                for rep in range(_env("K_WARM_REP", 5)):
                    nc.tensor.matmul(pw[:], dum[:, 0:128], dum[:],
                                     start=(rep == 0), stop=True)
            proj_group(k_tiles, wk_sb, xk0, 0, 0, psA, "pa")
            proj_group(q_tiles, wq_sb, xq0, 0, 0, psA, "pa")

        # ---------------- Phase B: attention + out-proj ----------------
        QB = [(0, 512, 2), (512, 512, 2), (1024, 512, 2),
              (1536, 256, 4), (1792, 256, 4)]
        NQB = len(QB)
        with tc.tile_pool(name="ev", bufs=_env("K_EV_BUFS", 7)) as ev, \
             tc.tile_pool(name="x", bufs=2) as xpool, \
             tc.tile_pool(name="xn", bufs=_env("K_XN_BUFS", 2)) as xnpool, \
             tc.tile_pool(name="small", bufs=_env("K_SMALL_BUFS", 4)) as small, \
             tc.tile_pool(name="o", bufs=2) as opool, \
             tc.tile_pool(name="psS", bufs=_env("K_PSS_BUFS", 3), space="PSUM") as psS, \
             tc.tile_pool(name="psX", bufs=_env("K_XO_BUFS", 2), space="PSUM") as psX:
            x_tiles = [xpool.tile([128, MT, 512], BF16, tag="xs",
                                  name=f"xs{i}") for i in range(2)]
            xn_stage = [xnpool.tile([128, 4, 512], BF16, tag="xn",
                                    name=f"xn{i}")
                        for i in range(_env("K_XN_BUFS", 2))]
            o_halves = {}   # (oqb, m//4) -> ([128,4,512] tile, writes-left)

            def outproj_group(oqb, m, flush=False):
                col0, W, _ = QB[oqb]
                x_prev = x_tiles[oqb % 2]
                hkey = (oqb, m // 4)
                if hkey not in o_halves:
                    o_halves[hkey] = [opool.tile([128, 4, 512], BF16,
                                                 tag="ob",
                                                 name=f"ob{oqb}_{m // 4}"), 4]
                o_sb, _ = o_halves[hkey]
                po = psS.tile([128, W], F32, tag="s", name=f"po{oqb}_{m}")
                for kk in range(MT):
                    nc.tensor.matmul(
                        po[:], wo_t[:, kk, m * 128:(m + 1) * 128],
                        x_prev[:, kk, 0:W], start=(kk == 0), stop=(kk == MT - 1))
                nc.vector.tensor_copy(o_sb[:, m % 4, 0:W], po[:])
                o_halves[hkey][1] -= 1
                if o_halves[hkey][1] == 0:
                    mh = (m // 4) * 4
                    nc.sync.dma_start(
                        outv[:, mh:mh + 4, col0:col0 + W], o_sb[:, :, 0:W])

            xts_store = {("v", n): xv_blocks[n] for n in range(NB)}
            xts_store[("q", 0)] = xq0

            def mk_vp(nn, sm, p):
                return ("mm", lambda: v_group_pair(nn, sm, p, psS, "s"))

            def mk_kg(m, b):
                return ("mm", lambda: proj_group(k_tiles, wk_sb, xk_blocks[b],
                                                 b, m, psS, "s"))

            def mk_qdma(nn):
                def f():
                    xts_store[("q", nn)] = dma_block(xqv, nn, "xq")
                return ("dma", f)

            def mk_qg(nn, m):
                return ("mm", lambda: proj_group(q_tiles, wq_sb,
                                                 xts_store[("q", nn)],
                                                 nn, m, psS, "s"))

            def mk_og(oqb, m, flush=False):
                return ("mm", lambda: outproj_group(oqb, m, flush))

            # (qb, p) -> [(min_sg, (kind, fn)), ...]
            # Block 0 hosts all remaining K m-tiles, per-pair V units and Q0
            # m-tiles, deadline-ordered: scores(p, sg) needs K m_p b(sg//2);
            # attnV at sg eats V t=sgw*(sg-LAG); pair p+1 needs K m_{p+1} b0
            # and Q0 m_{p+1} before it starts.
            side_work = {}

            def vp_sched(p, host_pair):
                """V units for pair p spread over hosting pair's sgs."""
                out = []
                for t in range(KT):
                    if host_pair == p:      # own pair: stay LAG sgs ahead
                        ms = max(1, t // 2)
                    else:                   # previous pair hosts: spread
                        ms = min(7, t // 2)
                    out.append((ms, mk_vp(t // 4, t % 4, p)))
                return out

            side_work[(0, 0)] = ([(1, mk_kg(0, 1)), (3, mk_kg(0, 2)),
                                  (5, mk_kg(0, 3)), (7, mk_kg(1, 0)),
                                  (7, mk_qg(0, 1))]
                                 + vp_sched(0, 0))
            side_work[(0, 1)] = ([(1, mk_kg(1, 1)), (3, mk_kg(1, 2)),
                                  (5, mk_kg(1, 3)), (7, mk_kg(2, 0)),
                                  (7, mk_qg(0, 2))]
                                 + vp_sched(1, 1))
            side_work[(0, 2)] = ([(2, mk_qdma(1)), (1, mk_kg(2, 1)),
                                  (3, mk_kg(2, 2)), (5, mk_kg(2, 3)),
                                  (7, mk_kg(3, 0)), (7, mk_qg(0, 3))]
                                 + vp_sched(2, 2))
            side_work[(0, 3)] = ([(1, mk_kg(3, 1)), (3, mk_kg(3, 2)),
                                  (5, mk_kg(3, 3)), (7, mk_qg(1, 0))]
                                 + vp_sched(3, 3))
            SIDE = {
                (1, 0): [(0, 'qdma', 2), (1, 'qg', 1, 1), (4, 'og', 0, 0),
                         (7, 'og', 0, 1), (4, 'qg', 2, 0)],
                (1, 1): [(0, 'qg', 1, 2), (3, 'og', 0, 2), (7, 'og', 0, 3),
                         (4, 'qg', 2, 1)],
                (1, 2): [(0, 'qg', 1, 3), (3, 'og', 0, 4), (7, 'og', 0, 5),
                         (4, 'qg', 2, 2)],
                (1, 3): [(3, 'og', 0, 6), (7, 'og', 0, 7), (4, 'qg', 2, 3)],
                (2, 0): [(0, 'qdma', 3), (4, 'og', 1, 0), (7, 'og', 1, 1),
                         (4, 'qg', 3, 0)],
                (2, 1): [(3, 'og', 1, 2), (7, 'og', 1, 3), (4, 'qg', 3, 1)],
                (2, 2): [(3, 'og', 1, 4), (7, 'og', 1, 5), (4, 'qg', 3, 2)],
                (2, 3): [(3, 'og', 1, 6), (7, 'og', 1, 7), (4, 'qg', 3, 3)],
                (3, 0): [(4, 'og', 2, 0)],
                (3, 1): [(1, 'og', 2, 2), (2, 'og', 2, 3)],
                (3, 2): [(1, 'og', 2, 4), (3, 'og', 2, 1)],
                (3, 3): [(1, 'og', 2, 5)],
                (4, 0): [(0, 'og', 2, 6), (2, 'og', 2, 7)],
                (4, 1): [(1, 'og', 3, 0), (2, 'og', 3, 1), (3, 'og', 3, 2)],
                (4, 2): [(0, 'og', 3, 3), (1, 'og', 3, 4), (3, 'og', 3, 5)],
                (4, 3): [(0, 'og', 3, 6), (1, 'og', 3, 7)],
            }
            for key, items in SIDE.items():
                lst = side_work.setdefault(key, [])
                for it in items:
                    if it[1] == 'qdma':
                        lst.append((it[0], mk_qdma(it[2])))
                    elif it[1] == 'qg':
                        lst.append((it[0], mk_qg(it[2], it[3])))
                    else:
                        lst.append((it[0], mk_og(it[2], it[3],
                                                 flush=(it[3] == DT - 1))))
            for key in side_work:
                side_work[key].sort(key=lambda it: it[0])

            MAXMM = _env("K_MAXMM", 1)
            MAXMM0 = _env("K_MAXMM0", 5)

            def side_step(qb, p, sg):
                work = side_work.get((qb, p))
                if not work:
                    return
                lim = MAXMM0 if qb == 0 else MAXMM
                did_mm = 0
                while work:
                    min_sg, (kind, fn) = work[0]
                    if min_sg > sg or (kind == "mm" and did_mm >= lim):
                        break
                    work.pop(0)
                    fn()
                    if kind == "mm":
                        did_mm += 1

            def side_flush(qb, p):
                for _, (kind, fn) in side_work.pop((qb, p), []):
                    fn()

            def attn_v(ps_x, h, sg, sgw, e_h, nqc):
                """Flipped attnV for supergroup sg: e chunks stationary,
                V [128, 65] moving, accumulating x~[q, hd|den] per qc."""
                for tt in range(sgw):
                    t = sg * sgw + tt
                    for qc in range(nqc):
                        # start=True zeroes the WHOLE psum bank, so only the
                        # very first matmul into this tile may set it; the
                        # other qc groups accumulate onto the zeroed bank.
                        nc.tensor.matmul(
                            ps_x[h][:, qc, :],
                            e_h[:, tt, qc * 128:(qc + 1) * 128],
                            v_sb[:, t, h, :],
                            start=(t == 0 and qc == 0), stop=(t == KT - 1))

            for qb in range(NQB):
                col0, W, sgw = QB[qb]
                nb = col0 // 512
                q0 = col0 % 512
                nsg = KT // sgw
                nqc = W // 128
                xn_sb = xn_stage[qb % len(xn_stage)]
                for p in range(MT):        # head pairs; pair p = heads 2p,2p+1
                    heads = (2 * p, 2 * p + 1)
                    ps_x = {h: psX.tile([128, nqc, DK + 1], F32, tag="xo",
                                        name=f"psx{qb}_{h}") for h in heads}
                    e_hist = {}
                    for sg in range(nsg):
                        side_step(qb, p, sg)
                        for h in heads:
                            hp = h % 2
                            ps_h = psS.tile([128, sgw, W], F32, tag="s",
                                            name=f"pss{qb}_{sg}_{h}")
                            for tt in range(sgw):
                                t = sg * sgw + tt
                                nc.tensor.matmul(
                                    ps_h[:, tt, :],
                                    k_tiles[(p, t // 4)][
                                        hp * 64:(hp + 1) * 64,
                                        (t % 4) * 128:(t % 4 + 1) * 128],
                                    q_tiles[(p, nb)][hp * 64:(hp + 1) * 64,
                                                     q0:q0 + W],
                                    start=True, stop=True)
                            e_sb = ev.tile([128, sgw, W], BF16, tag="e",
                                           name=f"e{qb}_{sg}_{h}")
                            nc.scalar.activation(e_sb[:], ps_h[:], EXP,
                                                 scale=float(SCALE))
                            e_hist[(sg, h)] = e_sb
                            if sg >= LAG:
                                attn_v(ps_x, h, sg - LAG, sgw,
                                       e_hist.pop((sg - LAG, h)), nqc)
                    side_flush(qb, p)
                    # drain last LAG supergroups, then normalize per
                    # q-chunk and flip each chunk back to [hd, q] via the
                    # DMA xbar transpose (SP queue + DMA engines, no PE
                    # cost) as soon as both heads' normalize lands, so the
                    # next block's outproj deps resolve early.
                    for h in heads:
                        for j in range(LAG, 0, -1):
                            attn_v(ps_x, h, nsg - j, sgw,
                                   e_hist.pop((nsg - j, h)), nqc)
                    for qc in range(nqc):
                        for h in heads:
                            hp = h % 2
                            c0 = p * 128 + hp * 64
                            r = small.tile([128, 1], F32, tag="r",
                                           name=f"r{qb}_{h}_{qc}")
                            nc.vector.reciprocal(r[:], ps_x[h][:, qc, DK:DK + 1])
                            nc.vector.tensor_scalar_mul(
                                xn_sb[:, qc, c0:c0 + 64],
                                ps_x[h][:, qc, 0:DK], r[:])
                        nc.sync.dma_start_transpose(
                            x_tiles[qb % 2][:, p, qc * 128:(qc + 1) * 128],
                            xn_sb[:, qc, p * 128:(p + 1) * 128])
            # final out-projection for the last q block (its og side-work
            # can't ride a following block).
            oqb = NQB - 1
            col0, W, _ = QB[oqb]
            x_prev = x_tiles[oqb % 2]
            for half in range(2):
                o_sb = opool.tile([128, 4, 512], BF16, tag="ob",
                                  name=f"obf{half}")
                for mm in range(4):
                    m = half * 4 + mm
                    ms = slice(m * 128, (m + 1) * 128)
                    po = psS.tile([128, W], F32, tag="s", name=f"pof{m}")
                    for kk in range(MT):
                        nc.tensor.matmul(
                            po[:], wo_t[:, kk, ms], x_prev[:, kk, 0:W],
                            start=(kk == 0), stop=(kk == MT - 1))
                    if m % 2:
                        nc.scalar.copy(o_sb[:, mm, 0:W], po[:])
                    else:
                        nc.vector.tensor_copy(o_sb[:, mm, 0:W], po[:])
                    if m == 6:
                        nc.sync.dma_start(
                            outv[:, 4:7, col0:col0 + W], o_sb[:, 0:3, 0:W])
                if half == 0:
                    nc.sync.dma_start(
                        outv[:, 0:4, col0:col0 + W], o_sb[:, :, 0:W])
                else:
                    nc.sync.dma_start(
                        outv[:, 7:8, col0:col0 + W], o_sb[:, 3:4, 0:W])
    nc.finalize()
    return nc


def _mmaj(wT):
    """[DM, CW] -> m-major [128, MT, DT, 128]: [p, m, k, j] = wT[k*128+p,
    m*128+j]."""
    return np.ascontiguousarray(
        wT.reshape(DT, 128, MT, 128).transpose(1, 2, 0, 3))


def kernel(query, key, value, mask, W_q, W_k, W_v, W_o):
    global _NC
    if _NC is None:
        _NC = _build()
    bf = ml_dtypes.bfloat16
    query = np.asarray(query, dtype=np.float32)
    key = np.asarray(key, dtype=np.float32)
    value = np.asarray(value, dtype=np.float32)
    W_q = np.asarray(W_q, dtype=np.float32)
    W_k = np.asarray(W_k, dtype=np.float32)
    W_v = np.asarray(W_v, dtype=np.float32)
    W_o = np.asarray(W_o, dtype=np.float32)
    mask = np.asarray(mask)

    in_maps = []
    for c in range(NC_CORES):
        b, g = divmod(c, 2)
        hs = slice(g * CW, (g + 1) * CW)
        mrow = (mask[b, 0, 0, :] != 0).astype(np.float32)
        in_maps.append({
            "xqT": np.ascontiguousarray(query[b].T).astype(bf),
            "xkT": np.ascontiguousarray(key[b].T).astype(bf),
            "xvT": np.ascontiguousarray(value[b].T).astype(bf),
            "wqT": _mmaj(W_q[hs, :].T).astype(bf),
            "wkT": _mmaj(W_k[hs, :].T).astype(bf),
            "wvT": _mmaj(W_v[hs, :].T).astype(bf),
            "woT": np.ascontiguousarray(W_o[:, hs].T).astype(bf),
            "maskf": np.ascontiguousarray(mrow.reshape(KT, 128).T),
        })
    res = run_bass_kernel_spmd(_NC, in_maps, core_ids=list(range(NC_CORES)))
    out = np.empty((B, S, DM), np.float32)
    for b in range(B):
        out[b] = (res.results[2 * b]["outT"].astype(np.float32)
                  + res.results[2 * b + 1]["outT"].astype(np.float32)).T
    return out
